# revision 28
# baseline (speedup 1.0000x reference)
"""Trainium2 Bass kernel for the GNN message-passing model.

Strategy: pure data-parallel over batch (B=16 -> 2 batches per core, 8 cores,
no cross-core communication). Activations are feature-major
([feat, batch*node]) for the per-node matmuls. The adjacency matmul keeps the
FULL adjacency SBUF-resident as fp8 e4m3 (adj^T * 4096, 16.8 MB) and runs in
DoubleRow fp8 perf mode (2 contraction rows/cycle): stationary operand is the
node-major cut-feature tile (fp8, scaled by 16), moving operand is a pair of
adj^T j-tiles. The node-major f_cut is produced directly by a second small
matmul per 128-node tile (stationary = x-slice, moving = cut columns of the
layer weight), avoiding PE transposes + PSUM casts. The 1/(4096*16) descale
is folded into the post-adjacency activation's scale.

Weight-only folds done on host (pure parameter preprocessing):
  W3fold = pw3 @ gw0[100:200]   (positional-MLP last layer folded into gw0)
  t4     = emb @ gw0[200:300]   (embedding table folded into gw0)
  pb3f   = pb3 @ gw0[100:200]   (bias fold)
mask_idx is re-encoded as a one-hot (4 classes, exact in fp8) so the
embedding lookup becomes a K=4 matmul accumulated into the same PSUM.
"""

import numpy as np
import ml_dtypes

import concourse.bass as bass
import concourse.mybir as mybir
import concourse.tile as tile
import concourse.bass_utils as _bass_utils
from concourse.bass_utils import run_bass_kernel_spmd

# (walrus's --enable-ldw-opt pass was tried and rejects this kernel's
# DoubleRow self-loading matmuls; keep the default.)

F32 = mybir.dt.float32
BF16 = mybir.dt.bfloat16
F8 = mybir.dt.float8e4
AF = mybir.ActivationFunctionType
DR = mybir.MatmulPerfMode.DoubleRow
BF = ml_dtypes.bfloat16
E4 = ml_dtypes.float8_e4m3

B, N, BC = 16, 4096, 2          # batches, nodes, batches per core
NCORES = 8
NB = N // 512                   # 8 i-blocks of 512 output nodes
JT = N // 128                   # 32 contraction j-tiles
MAGIC = float(1.5 * 2 ** 23)    # fp32 round-to-nearest magic constant
TWO_PI = float(2.0 * np.pi)
ASCALE = 4096.0                 # adj premultiplier (host, before fp8 cast)
FSCALE = 16.0                   # f_cut premultiplier (device, before fp8)
OSCALE = float(1.0 / (ASCALE * FSCALE))

run_kwargs = {}                 # test.py may inject trace kwargs here


def split_excess_waits(nc, max_waits=1):
    """Walrus codegen on this image rejects >1 sem wait per instruction;
    move excess waits onto preceding same-engine no-ops."""
    n_split = 0
    for fn in nc.m.functions:
        for blk in fn.blocks:
            insts = list(blk.instructions)
            out = []
            changed = False
            for inst in insts:
                si = getattr(inst, "sync_info", None)
                if si is not None and len(si.on_wait) > max_waits:
                    waits = list(si.on_wait)
                    chunks = [waits[i:i + max_waits]
                              for i in range(0, len(waits), max_waits)]
                    for ci, ch in enumerate(chunks[:-1]):
                        nop = mybir.InstNoOp(
                            name=f"{inst.name}-wsplit-{ci}", ins=[], outs=[])
                        nop.engine = inst.engine
                        nop.sync_info = mybir.SyncInfo(on_wait=ch, on_update=[])
                        out.append(nop)
                        n_split += 1
                    inst.sync_info = mybir.SyncInfo(
                        on_wait=chunks[-1], on_update=list(si.on_update))
                    changed = True
                out.append(inst)
            if changed:
                blk.instructions = out
    return n_split


def _param(nc, name, shape, dt):
    return nc.declare_dram_parameter(name, list(shape), dt, isOutput=False)


def build_bass(split=True):
    nc = bass.Bass()

    adjT8 = _param(nc, "adjT8", [N, N], F8)
    meshHLd = _param(nc, "meshHLd", [14, N], BF16)
    onehotd = _param(nc, "onehotd", [BC, 64, N], BF16)
    maskTd = _param(nc, "maskTd", [50, BC], F32)

    pw1p = _param(nc, "pw1p", [67, 25], BF16)
    pw2d = _param(nc, "pw2d", [25, 50], BF16)
    w3t4d = _param(nc, "w3t4d", [114, 128], BF16)
    gw1d = _param(nc, "gw1d", [128, 128], BF16)
    gw2d = _param(nc, "gw2d", [128, 128], BF16)
    gw3d = _param(nc, "gw3d", [128, 50], BF16)
    aw1ad = _param(nc, "aw1ad", [50, 128], F32)
    aw1bd = _param(nc, "aw1bd", [50, 72], F32)
    aw2ad = _param(nc, "aw2ad", [128, 100], F32)
    aw2bd = _param(nc, "aw2bd", [72, 100], F32)
    aw3d = _param(nc, "aw3d", [100, 100], F32)
    gw0Ld = _param(nc, "gw0Ld", [100, 128], F32)
    pb3frd = _param(nc, "pb3frd", [1, 128], BF16)
    selfAd = _param(nc, "selfAd", [14, 128], BF16)
    selfBd = _param(nc, "selfBd", [14, 62], BF16)
    biasd = _param(nc, "biasd", [128, 12], F32)
    # bias columns (within biasd): 0 ab1a[128], 1 ab1b[72], 2 ab2[100],
    # 3 ab3[100], 4 pb3f[128], 5 pb1[25], 6 pb2[50], 7 gbl0[x2@0/64],
    # 8 gbl1, 9 gbl2, 10 gb3x2[114]
    outd = nc.declare_dram_parameter("outd", [114, 1], F32, isOutput=True)

    with tile.TileContext(nc) as tc:
        _emit(nc, tc, locals())
    if split:
        split_excess_waits(nc)
    return nc


def _emit(nc, tc, d):
    import contextlib
    ctx = contextlib.ExitStack()
    meshHLd, onehotd, maskTd = d["meshHLd"], d["onehotd"], d["maskTd"]
    biasd, outd = d["biasd"], d["outd"]

    cpool = ctx.enter_context(tc.tile_pool(name="consts", bufs=1))
    resp = ctx.enter_context(tc.tile_pool(name="resadj", bufs=1))
    actp = ctx.enter_context(tc.tile_pool(name="acts", bufs=1))
    smallp = ctx.enter_context(tc.tile_pool(name="small", bufs=2))
    dvep = ctx.enter_context(tc.tile_pool(name="dvework", bufs=3))
    h1p = ctx.enter_context(tc.tile_pool(name="h1p", bufs=2))

    # PSUM budget (8 banks): "feat"x2 + "bp"x2 + left0..left3 x1
    ps_a = ctx.enter_context(tc.tile_pool(name="psa", bufs=2, space="PSUM"))
    ps_b = ctx.enter_context(tc.tile_pool(name="psb", bufs=2, space="PSUM"))
    ps_c = ctx.enter_context(tc.tile_pool(name="psc", bufs=1, space="PSUM"))

    # ---------------- constants (small, issue before the big adj DMAs) ----
    def ctile(dram, shape, dt):
        nm = f"c_{dram.name}"
        t = cpool.tile(list(shape), dt, tag=nm, name=nm)
        nc.sync.dma_start(out=t[:], in_=dram[:])
        return t

    pw1 = ctile(d["pw1p"], [67, 25], BF16)
    pw2 = ctile(d["pw2d"], [25, 50], BF16)
    w3t4 = ctile(d["w3t4d"], [114, 128], BF16)
    gws = [None, ctile(d["gw1d"], [128, 128], BF16),
           ctile(d["gw2d"], [128, 128], BF16),
           ctile(d["gw3d"], [128, 50], BF16)]
    aw1a = ctile(d["aw1ad"], [50, 128], F32)
    aw1b = ctile(d["aw1bd"], [50, 72], F32)
    aw2a = ctile(d["aw2ad"], [128, 100], F32)
    aw2b = ctile(d["aw2bd"], [72, 100], F32)
    aw3 = ctile(d["aw3d"], [100, 100], F32)
    gw0L = ctile(d["gw0Ld"], [100, 128], F32)
    pb3fr = ctile(d["pb3frd"], [1, 128], BF16)
    selfA = ctile(d["selfAd"], [14, 128], BF16)
    selfB = ctile(d["selfBd"], [14, 62], BF16)
    biases = ctile(biasd, [128, 12], F32)
    maskT = ctile(maskTd, [50, BC], F32)
    ones1 = cpool.tile([1, 128], BF16, tag="ones1", name="ones1")
    nc.vector.memset(ones1[:], 1.0)

    def bcol(col, p0, p1):
        return biases[p0:p1, col:col + 1]

    # ---------------- resident adj^T (fp8, full) ----------------
    adjbig = resp.tile([128, JT * N], F8, tag="adj", name="adjbig")
    adj3 = adjbig[:].rearrange("p (q n) -> p q n", n=N)
    adjr = d["adjT8"][:].rearrange("(q p) c -> p q c", p=128)  # [128,32,4096]
    for q0 in range(0, JT, 4):
        nc.sync.dma_start(out=adj3[:, q0:q0 + 4, :], in_=adjr[:, q0:q0 + 4, :])

    # ---------------- activation tiles ----------------
    xt = actp.tile([128, BC * N], BF16, tag="x")          # [feat, b*N+n]
    # fcst padding columns 42:64 / 114:128 only feed PSUM partitions that
    # are never read back, but keep them zeroed: zero stationary columns
    # toggle less PE logic than garbage bits.
    fcst = actp.tile([128, JT * 128], F8, tag="fcst")     # node-major f_cut
    nc.scalar.memzero(fcst[:])
    fc3 = fcst[:].rearrange("p (q n) -> p q n", n=128)    # [128, 32, 128]
    # h2o: rows 0:4 one-hot, rows 4:64 zeros (alignment padding),
    # rows 64:114 h2 — one contraction for the layer-0 feature matmuls
    h2o = [actp.tile([114, N], BF16, tag=f"h2o{b}", name=f"h2o{b}")
           for b in range(BC)]
    nc.gpsimd.dma_start(out=h2o[0][0:64, :], in_=onehotd[0, :, :])
    nc.scalar.dma_start(out=h2o[1][0:64, :], in_=onehotd[1, :, :])
    cvec = actp.tile([128, BC], F32, tag="cvec")
    cvb = [actp.tile([1, 128], BF16, tag=f"cvb{b}", name=f"cvb{b}")
           for b in range(BC)]
    mx = actp.tile([114, NB], F32, tag="mx")
    outsb = actp.tile([114, 1], F32, tag="outsb")

    # ---------------- action MLP (tiny, fp32) ----------------
    pa = ps_a.tile([128, 2], F32, tag="feat")
    nc.tensor.matmul(pa[:], lhsT=aw1a[:], rhs=maskT[:], start=True, stop=True)
    a1a = smallp.tile([128, 2], F32, tag="a1a")
    nc.scalar.activation(a1a[:], pa[:], AF.Relu, bias=bcol(0, 0, 128))
    pb = ps_a.tile([72, 2], F32, tag="feat")
    nc.tensor.matmul(pb[:], lhsT=aw1b[:], rhs=maskT[:], start=True, stop=True)
    a1b = smallp.tile([72, 2], F32, tag="a1b")
    nc.scalar.activation(a1b[:], pb[:], AF.Relu, bias=bcol(1, 0, 72))
    pc = ps_a.tile([100, 2], F32, tag="feat")
    nc.tensor.matmul(pc[:], lhsT=aw2a[:], rhs=a1a[:], start=True, stop=False)
    nc.tensor.matmul(pc[:], lhsT=aw2b[:], rhs=a1b[:], start=False, stop=True)
    a2 = smallp.tile([100, 2], F32, tag="a2")
    nc.scalar.activation(a2[:], pc[:], AF.Relu, bias=bcol(2, 0, 100))
    pd = ps_a.tile([100, 2], F32, tag="feat")
    nc.tensor.matmul(pd[:], lhsT=aw3[:], rhs=a2[:], start=True, stop=True)
    a3 = smallp.tile([100, 2], F32, tag="a3")
    nc.scalar.activation(a3[:], pd[:], AF.Identity, bias=bcol(3, 0, 100))
    # cvec[f, b] = (a3 @ gw0[:100,:]) + pb3f  (bias for layer-0 features)
    pe_ = ps_a.tile([128, 2], F32, tag="feat")
    nc.tensor.matmul(pe_[:], lhsT=gw0L[:], rhs=a3[:], start=True, stop=True)
    nc.scalar.activation(cvec[:], pe_[:], AF.Identity, bias=bcol(4, 0, 128))
    # row versions cvb[b] = cvec[:, b].T for the node-major layer-0 matmul
    for b in range(BC):
        pr = ps_b.tile([1, 128], F32, tag="bp")
        nc.tensor.matmul(pr[:], lhsT=a3[:, b:b + 1], rhs=gw0L[:],
                         start=True, stop=False)
        nc.tensor.matmul(pr[:], lhsT=ones1[0:1, 0:1], rhs=pb3fr[:],
                         start=False, stop=True)
        nc.vector.tensor_copy(cvb[b][:], pr[:])
    # w3t4c[b] = w3t4 with the one-hot rows bumped by cvb[b]: folds the
    # action-embedding bias into the node-major layer-0 matmul
    w3t4c = [actp.tile([114, 128], BF16, tag=f"w3t4c{b}", name=f"w3t4c{b}")
             for b in range(BC)]
    for b in range(BC):
        pt4 = ps_b.tile([4, 128], F32, tag="bp")
        nc.tensor.matmul(pt4[:], lhsT=ones1[0:1, 0:4], rhs=cvb[b][0:1, :],
                         start=True, stop=True)
        nc.vector.tensor_copy(w3t4c[b][:], w3t4[:])
        nc.vector.tensor_add(w3t4c[b][0:4, :], w3t4[0:4, :], pt4[:])

    # ---------------- GCN layers ----------------
    def phase_bp(li, jt0, jt1):
        """Produce node-major fcst (fp8, x FSCALE): fc3[p, jt, 64b+c]."""
        cd = 42 if li < 3 else 50
        grp = 504 // cd  # jt groups per PSUM bank
        for b in range(BC):
            jt = jt0
            while jt < jt1:
                ng = min(grp, jt1 - jt)
                pg = ps_b.tile([128, grp * cd], F32, tag="bp")
                pg3 = pg[:].rearrange("p (g c) -> p g c", c=cd)
                for g in range(ng):
                    jc = slice((jt + g) * 128, (jt + g) * 128 + 128)
                    if li == 0:
                        nc.tensor.matmul(pg3[:, g, :], lhsT=h2o[b][:, jc],
                                         rhs=w3t4c[b][:, 0:cd],
                                         start=True, stop=True)
                    else:
                        nc.tensor.matmul(pg3[:, g, :],
                                         lhsT=xt[:, b * N + (jt + g) * 128:
                                                 b * N + (jt + g) * 128 + 128],
                                         rhs=gws[li][:, 0:cd],
                                         start=True, stop=True)
                dst = fc3[:, jt:jt + ng, 64 * b:64 * b + cd]
                src = pg3[:, 0:ng, :]
                if b == 0:
                    nc.vector.tensor_scalar_mul(dst, src, FSCALE)
                else:
                    nc.scalar.activation(dst, src, AF.Identity, scale=FSCALE)
                jt += ng

    def phase_a(li):
        """Feature-major right part: xt[32:128] = relu(f[32:128]).
        Chunk-major order: chunks 0-3 only need the first phase-C half's
        activations, hiding the second half's act drain at layer entry."""
        for ch in range(NB):
            for b in range(BC):
                xs = slice(b * N + ch * 512, b * N + (ch + 1) * 512)
                pf = ps_a.tile([128, 512], F32, tag="feat")
                nc.tensor.matmul(pf[:], lhsT=gws[li][:],
                                 rhs=xt[:, xs], start=True, stop=True)
                nc.vector.tensor_scalar_max(xt[32:64, xs], pf[32:64, :],
                                            0.0)
                nc.scalar.activation(xt[64:128, xs], pf[64:128, :],
                                     AF.Relu)

    def c_alloc(half):
        return {ib: ps_c.tile([114, 512], F32, tag=f"left{ib % 4}",
                              name=f"left{ib % 4}") for ib in half}

    def c_half(li, pls, t0, t1):
        cd = 42 if li < 3 else 50
        W = 64 + cd
        for t in range(t0, t1):
            for ib in pls:
                nc.tensor.matmul(
                    pls[ib][0:W, :],
                    lhsT=fc3[:, 2 * t:2 * t + 2, 0:W],
                    rhs=adj3[:, 2 * t:2 * t + 2,
                             ib * 512:(ib + 1) * 512],
                    start=(t == 0), stop=(t == JT // 2 - 1),
                    perf_mode=DR)

    def c_acts(li, pls):
        if True:
            for ib in pls:
                if li < 3:
                    # xt[0:42] keeps the 2^16 (ASCALE*FSCALE) scale; the
                    # next layer's gw rows 0:42 are descaled on the host and
                    # the gb biases prescaled, so relu needs no scale here.
                    xs0 = slice(ib * 512, (ib + 1) * 512)
                    xs1 = slice(N + ib * 512, N + (ib + 1) * 512)
                    nc.vector.tensor_scalar(
                        xt[0:42, xs0], pls[ib][0:42, :],
                        bcol(7 + li, 0, 42), 0.0, ADD, MAX)
                    nc.scalar.activation(
                        xt[0:42, xs1], pls[ib][64:106, :], AF.Relu,
                        bias=bcol(7 + li, 64, 106))
                else:
                    # one 114-partition reduce; rows 50:64 are garbage but
                    # never read back on the host
                    nc.vector.tensor_reduce(
                        mx[:, ib:ib + 1], pls[ib][:, :],
                        mybir.AxisListType.X, mybir.AluOpType.max)


    # ---------------- positional front-end (+ layer-0 A/B\' interleave) ---
    # m6 rows: 0:6 mesh hi (b,c), 6:12 mesh lo, 12 = 0.25, 13 = 1.0 (all
    # host-built, one DMA).  psA rows 0:62 = t, rows 64:126 = t + MAGIC (the
    # PE accumulates the row-13 MAGIC term last, rounding t to the nearest
    # integer in fp32).  psB = (t + 0.25) + MAGIC.
    ADD, MAX = mybir.AluOpType.add, mybir.AluOpType.max
    peins = []
    for i in range(4):
        pt = cpool.tile([67, 512], BF16, tag=f"pein{i}", name=f"pein{i}")
        nc.vector.memset(pt[:], 0.0)
        peins.append(pt)
    m6s = [cpool.tile([14, 512], BF16, tag=f"m6_{i}", name=f"m6_{i}")
           for i in range(3)]
    # prefetch the first two mesh chunks before entering the loop
    for k in range(2):
        nc.gpsimd.dma_start(out=m6s[k][:],
                            in_=meshHLd[:, k * 512:(k + 1) * 512])

    def a0_chunk(b, ch):
        cs = slice(ch * 512, (ch + 1) * 512)
        xs = slice(b * N + ch * 512, b * N + (ch + 1) * 512)
        # borrow the (idle during phase 1) phase-C banks so the layer-0
        # feature matmuls don't serialize against the ph1/ph2 rotation
        pf = ps_c.tile([128, 512], F32, tag=f"left{b}", name=f"left{b}")
        nc.tensor.matmul(pf[:], lhsT=w3t4[:], rhs=h2o[b][:, cs],
                         start=True, stop=True)
        nc.vector.tensor_scalar(xt[32:64, xs], pf[32:64, :],
                                cvec[32:64, b:b + 1], 0.0, ADD, MAX)
        nc.scalar.activation(xt[64:128, xs], pf[64:128, :],
                             AF.Relu, bias=cvec[64:128, b:b + 1])

    for ch in range(NB):
        cs = slice(ch * 512, (ch + 1) * 512)
        m6 = m6s[ch % 3]
        if ch + 2 < NB:
            nc.gpsimd.dma_start(
                out=m6s[(ch + 2) % 3][:],
                in_=meshHLd[:, (ch + 2) * 512:(ch + 3) * 512])
        psA = ps_b.tile([128, 512], F32, tag="bp")
        nc.tensor.matmul(psA[:], lhsT=selfA[:], rhs=m6[:], start=True,
                         stop=True)
        psB = ps_b.tile([62, 512], F32, tag="bp")
        nc.tensor.matmul(psB[:], lhsT=selfB[:], rhs=m6[:], start=True,
                         stop=True)
        # rr rows 0:62 = round(t); rows 64:126 = round(t+0.25) (exact:
        # Sterbenz cancellation of MAGIC)
        rr = dvep.tile([128, 512], F32, tag="rr")
        nc.vector.tensor_scalar_add(rr[0:62, :], psA[64:126, :], -MAGIC)
        # rows 64:126 = round(t+0.25) - 0.25 (both subtractions exact)
        nc.vector.tensor_scalar(rr[64:126, :], psB[:], -MAGIC, -0.25,
                                ADD, ADD)
        # ddc rows 0:62 = t - round(t); rows 64:126 = (t+.25) - round(t+.25)
        ddc = dvep.tile([128, 512], F32, tag="ddc")
        nc.vector.tensor_sub(ddc[0:62, :], psA[0:62, :], rr[0:62, :])
        nc.vector.tensor_sub(ddc[64:126, :], psA[0:62, :], rr[64:126, :])
        for b in range(BC):
            pein = peins[2 * b + ch % 2]
            nc.scalar.activation(pein[0:30, :], ddc[32 * b:32 * b + 30, :],
                                 AF.Sin, scale=TWO_PI)
            nc.scalar.activation(pein[32:62, :],
                                 ddc[64 + 32 * b:64 + 32 * b + 30, :],
                                 AF.Sin, scale=TWO_PI)
            nc.gpsimd.dma_start(out=pein[64:67, :],
                                in_=meshHLd[3 * b:3 * b + 3, cs])
            # h1 = relu(pe_in @ pw1 + pb1)
            ph1 = ps_a.tile([25, 512], F32, tag="feat")
            nc.tensor.matmul(ph1[:], lhsT=pw1[:], rhs=pein[:],
                             start=True, stop=True)
            h1t = h1p.tile([25, 512], BF16, tag="h1")
            nc.scalar.activation(h1t[:], ph1[:], AF.Relu, bias=bcol(5, 0, 25))
            # h2 = relu(h1 @ pw2 + pb2)  (relu+bias on DVE to unload scalar)
            ph2 = ps_a.tile([50, 512], F32, tag="feat")
            nc.tensor.matmul(ph2[:], lhsT=pw2[:], rhs=h1t[:],
                             start=True, stop=True)
            nc.vector.tensor_scalar(h2o[b][64:114, cs], ph2[:],
                                    bcol(6, 0, 50), 0.0, ADD, MAX)
        for b in range(BC):
            a0_chunk(b, ch)
        if ch == 2:
            phase_bp(0, 0, 12)
        elif ch == 5:
            phase_bp(0, 12, 24)
        elif ch == 7:
            phase_bp(0, 24, JT)

    for li in (0, 1, 2, 3):
        if 0 < li < 3:
            phase_a(li)
        phase_bp(li, 0, JT)
        pls = c_alloc(range(0, 4))
        c_half(li, pls, 0, JT // 2)
        c_acts(li, pls)
        pls = c_alloc(range(4, NB))
        c_half(li, pls, 0, JT // 2)
        c_acts(li, pls)

    # ---------------- final max + bias + output ----------------
    mxr = smallp.tile([114, 1], F32, tag="mxr")
    nc.vector.tensor_reduce(mxr[:], mx[:], mybir.AxisListType.X,
                            mybir.AluOpType.max)
    nc.scalar.activation(outsb[:], mxr[:], AF.Identity, bias=bcol(10, 0, 114),
                         scale=OSCALE)
    nc.sync.dma_start(out=outd[:], in_=outsb[:])
    ctx.close()


# ---------------------------------------------------------------------------
# host side
# ---------------------------------------------------------------------------

def _descale_gw(gw):
    """Rows 0:42 consume the 2^16-scaled adjacency output; descale (exact)."""
    g = gw.astype(np.float32).copy()
    g[0:42] *= np.float32(1.0 / (ASCALE * FSCALE))
    return g.astype(BF)


def _prep_shared(inp):
    """Host preprocessing shared across cores (weights + adj)."""
    f32 = np.float32
    adjT8 = (np.ascontiguousarray(inp["adj"].astype(f32).T)
             * np.float32(ASCALE)).astype(E4)

    gw0 = inp["gw0"].astype(f32)
    w3fold = (inp["pw3"].astype(f32) @ gw0[100:200]).astype(BF)
    t4 = (inp["emb"].astype(f32) @ gw0[200:300]).astype(BF)
    pb3f = (inp["pb3"].astype(f32) @ gw0[100:200]).astype(f32)
    w3t4 = np.zeros((114, 128), BF)
    w3t4[0:4] = t4
    w3t4[64:114] = w3fold

    # pe_in row permutation: ours = [sin(f,c) x30 | cos(f,c) x30 | mesh x3]
    pw1f = inp["pw1"].astype(f32)
    pw1p_ = np.zeros((67, 25), f32)
    for k in range(30):
        f, c = divmod(k, 3)
        pw1p_[k] = pw1f[f * 6 + c]          # sin rows
        pw1p_[32 + k] = pw1f[f * 6 + 3 + c]  # cos rows
    pw1p_[64:67] = pw1f[60:63]
    pw1p = pw1p_.astype(BF)

    freqs = np.asarray([np.pi] + [2.0 * np.pi * i for i in range(1, 10)], f32)
    freq2 = np.repeat(freqs, 3) / (2.0 * np.pi)   # [30]
    self6 = np.zeros((6, 62), f32)
    for b in range(2):
        for k in range(30):
            self6[3 * b + k % 3, 32 * b + k] = freq2[k]
    # m6 rows: 0:6 mesh hi, 6:12 mesh lo, 12 = 0.25, 13 = 1.0.  selfA maps
    # them to t (cols 0:62) and t + MAGIC (cols 64:126); selfB to
    # (t + 0.25) + MAGIC.  freq2 entries and MAGIC are bf16-exact; the PE
    # accumulates rows in order, so the MAGIC term lands last and rounds
    # t (resp. t + 0.25) to the nearest integer in the fp32 accumulator.
    selfA = np.zeros((14, 128), f32)
    selfA[0:6, 0:62] = self6
    selfA[0:6, 64:126] = self6
    selfA[6:12, 0:62] = self6
    selfA[6:12, 64:126] = self6
    selfA[13, 64:126] = np.float32(MAGIC)
    selfB = np.zeros((14, 62), f32)
    selfB[0:6, :] = self6
    selfB[6:12, :] = self6
    selfB[12, :] = 1.0
    selfB[13, :] = np.float32(MAGIC)

    biasd = np.zeros((128, 12), f32)
    biasd[0:128, 0] = inp["ab1"][:128]
    biasd[0:72, 1] = inp["ab1"][128:200]
    biasd[0:100, 2] = inp["ab2"]
    biasd[0:100, 3] = inp["ab3"]
    biasd[0:128, 4] = pb3f
    biasd[0:25, 5] = inp["pb1"].astype(f32)
    biasd[0:50, 6] = inp["pb2"].astype(f32)
    # gb biases for layers 0-2 prescaled by 2^16 = ASCALE*FSCALE: the
    # post-adjacency relu output keeps that scale, and the next layer's gw
    # rows 0:42 are descaled to compensate.
    XS = np.float32(ASCALE * FSCALE)
    for li in range(3):
        biasd[0:42, 7 + li] = inp[f"gb{li}"].astype(f32)[:42] * XS
        biasd[64:106, 7 + li] = inp[f"gb{li}"].astype(f32)[:42] * XS
    biasd[0:50, 10] = inp["gb3"].astype(f32)
    biasd[64:114, 10] = inp["gb3"].astype(f32)

    return {
        "adjT8": adjT8,
        "pw1p": pw1p,
        "pw2d": inp["pw2"].astype(BF),
        "w3t4d": w3t4,
        "gw1d": _descale_gw(inp["gw1"]),
        "gw2d": _descale_gw(inp["gw2"]),
        "gw3d": _descale_gw(inp["gw3"]),
        "aw1ad": np.ascontiguousarray(inp["aw1"].astype(f32)[:, :128]),
        "aw1bd": np.ascontiguousarray(inp["aw1"].astype(f32)[:, 128:200]),
        "aw2ad": np.ascontiguousarray(inp["aw2"].astype(f32)[:128]),
        "aw2bd": np.ascontiguousarray(inp["aw2"].astype(f32)[128:200]),
        "aw3d": inp["aw3"].astype(f32),
        "gw0Ld": np.ascontiguousarray(gw0[:100]),
        "pb3frd": pb3f.reshape(1, 128).astype(BF),
        "selfAd": selfA.astype(BF),
        "selfBd": selfB.astype(BF),
        "biasd": biasd,
    }


def _prep_core(inp, shared, core):
    bs = slice(core * BC, (core + 1) * BC)
    f32 = np.float32
    mesh = inp["mesh"].astype(f32)[bs]                       # [2, N, 3]
    meshT = mesh.transpose(0, 2, 1).reshape(6, N)            # rows (b,c)
    hi = meshT.astype(BF)
    lo = (meshT - hi.astype(f32)).astype(BF)
    meshHL = np.zeros((14, N), BF)
    meshHL[0:6] = hi
    meshHL[6:12] = lo
    meshHL[12] = BF(0.25)
    meshHL[13] = BF(1.0)
    mi = inp["mask_idx"][bs]                                 # [2, N] int32
    oh = (mi[:, None, :] == np.arange(4, dtype=mi.dtype)[None, :, None])
    onehot = np.zeros((BC, 64, N), BF)
    onehot[:, 0:4, :] = oh.astype(BF)
    maskT = np.ascontiguousarray(inp["mask"].astype(f32)[bs].T)  # [50, 2]
    m = dict(shared)
    m["meshHLd"] = meshHL
    m["onehotd"] = onehot
    m["maskTd"] = maskT
    return m


_CACHED = {}


def kernel(**inputs) -> np.ndarray:
    if "nc" not in _CACHED:
        _CACHED["nc"] = build_bass()
    nc = _CACHED["nc"]
    shared = _prep_shared(inputs)
    in_maps = [_prep_core(inputs, shared, c) for c in range(NCORES)]
    res = run_bass_kernel_spmd(nc, in_maps, list(range(NCORES)), **run_kwargs)
    out = np.empty((B, 50), np.float32)
    for c in range(NCORES):
        o = res.results[c]["outd"][:, 0]
        out[2 * c] = o[0:50]
        out[2 * c + 1] = o[64:114]
    _CACHED["last_results"] = res
    return out


# revision 30
# speedup vs baseline: 1.0023x; 1.0023x over previous
"""Trainium2 Bass kernel for the GNN message-passing model.

Strategy: pure data-parallel over batch (B=16 -> 2 batches per core, 8 cores,
no cross-core communication). Activations are feature-major
([feat, batch*node]) for the per-node matmuls. The adjacency matmul keeps the
FULL adjacency SBUF-resident as fp8 e4m3 (adj^T * 4096, 16.8 MB) and runs in
DoubleRow fp8 perf mode (2 contraction rows/cycle): stationary operand is the
node-major cut-feature tile (fp8, scaled by 16), moving operand is a pair of
adj^T j-tiles. The node-major f_cut is produced directly by a second small
matmul per 128-node tile (stationary = x-slice, moving = cut columns of the
layer weight), avoiding PE transposes + PSUM casts. The 1/(4096*16) descale
is folded into the post-adjacency activation's scale.

Weight-only folds done on host (pure parameter preprocessing):
  W3fold = pw3 @ gw0[100:200]   (positional-MLP last layer folded into gw0)
  t4     = emb @ gw0[200:300]   (embedding table folded into gw0)
  pb3f   = pb3 @ gw0[100:200]   (bias fold)
mask_idx is re-encoded as a one-hot (4 classes, exact in fp8) so the
embedding lookup becomes a K=4 matmul accumulated into the same PSUM.
"""

import numpy as np
import ml_dtypes

import concourse.bass as bass
import concourse.mybir as mybir
import concourse.tile as tile
import concourse.bass_utils as _bass_utils
from concourse.bass_utils import run_bass_kernel_spmd

# (walrus's --enable-ldw-opt pass was tried and rejects this kernel's
# DoubleRow self-loading matmuls; keep the default.)

F32 = mybir.dt.float32
BF16 = mybir.dt.bfloat16
F8 = mybir.dt.float8e4
AF = mybir.ActivationFunctionType
DR = mybir.MatmulPerfMode.DoubleRow
BF = ml_dtypes.bfloat16
E4 = ml_dtypes.float8_e4m3

B, N, BC = 16, 4096, 2          # batches, nodes, batches per core
NCORES = 8
NB = N // 512                   # 8 i-blocks of 512 output nodes
JT = N // 128                   # 32 contraction j-tiles
MAGIC = float(1.5 * 2 ** 23)    # fp32 round-to-nearest magic constant
TWO_PI = float(2.0 * np.pi)
ASCALE = 4096.0                 # adj premultiplier (host, before fp8 cast)
FSCALE = 16.0                   # f_cut premultiplier (device, before fp8)
OSCALE = float(1.0 / (ASCALE * FSCALE))

run_kwargs = {}                 # test.py may inject trace kwargs here


def split_excess_waits(nc, max_waits=1):
    """Walrus codegen on this image rejects >1 sem wait per instruction;
    move excess waits onto preceding same-engine no-ops."""
    n_split = 0
    for fn in nc.m.functions:
        for blk in fn.blocks:
            insts = list(blk.instructions)
            out = []
            changed = False
            for inst in insts:
                si = getattr(inst, "sync_info", None)
                if si is not None and len(si.on_wait) > max_waits:
                    waits = list(si.on_wait)
                    chunks = [waits[i:i + max_waits]
                              for i in range(0, len(waits), max_waits)]
                    for ci, ch in enumerate(chunks[:-1]):
                        nop = mybir.InstNoOp(
                            name=f"{inst.name}-wsplit-{ci}", ins=[], outs=[])
                        nop.engine = inst.engine
                        nop.sync_info = mybir.SyncInfo(on_wait=ch, on_update=[])
                        out.append(nop)
                        n_split += 1
                    inst.sync_info = mybir.SyncInfo(
                        on_wait=chunks[-1], on_update=list(si.on_update))
                    changed = True
                out.append(inst)
            if changed:
                blk.instructions = out
    return n_split


def _param(nc, name, shape, dt):
    return nc.declare_dram_parameter(name, list(shape), dt, isOutput=False)


def build_bass(split=True):
    nc = bass.Bass()

    adjT8 = _param(nc, "adjT8", [N, N], F8)
    meshHLd = _param(nc, "meshHLd", [14, N], BF16)
    onehotd = _param(nc, "onehotd", [BC, 64, N], BF16)
    maskTd = _param(nc, "maskTd", [50, BC], F32)

    pw1p = _param(nc, "pw1p", [67, 25], BF16)
    pw2d = _param(nc, "pw2d", [25, 50], BF16)
    w3t4d = _param(nc, "w3t4d", [114, 128], BF16)
    gw1d = _param(nc, "gw1d", [128, 128], BF16)
    gw2d = _param(nc, "gw2d", [128, 128], BF16)
    gw3d = _param(nc, "gw3d", [128, 50], BF16)
    aw1ad = _param(nc, "aw1ad", [50, 128], F32)
    aw1bd = _param(nc, "aw1bd", [50, 72], F32)
    aw2ad = _param(nc, "aw2ad", [128, 100], F32)
    aw2bd = _param(nc, "aw2bd", [72, 100], F32)
    aw3d = _param(nc, "aw3d", [100, 100], F32)
    gw0Ld = _param(nc, "gw0Ld", [100, 128], F32)
    pb3frd = _param(nc, "pb3frd", [1, 128], BF16)
    selfAd = _param(nc, "selfAd", [14, 128], BF16)
    selfBd = _param(nc, "selfBd", [14, 62], BF16)
    biasd = _param(nc, "biasd", [128, 12], F32)
    # bias columns (within biasd): 0 ab1a[128], 1 ab1b[72], 2 ab2[100],
    # 3 ab3[100], 4 pb3f[128], 5 pb1[25], 6 pb2[50], 7 gbl0[x2@0/64],
    # 8 gbl1, 9 gbl2, 10 gb3x2[114]
    outd = nc.declare_dram_parameter("outd", [114, 1], F32, isOutput=True)

    with tile.TileContext(nc) as tc:
        _emit(nc, tc, locals())
    if split:
        split_excess_waits(nc)
    return nc


def _emit(nc, tc, d):
    import contextlib
    ctx = contextlib.ExitStack()
    meshHLd, onehotd, maskTd = d["meshHLd"], d["onehotd"], d["maskTd"]
    biasd, outd = d["biasd"], d["outd"]

    cpool = ctx.enter_context(tc.tile_pool(name="consts", bufs=1))
    resp = ctx.enter_context(tc.tile_pool(name="resadj", bufs=1))
    actp = ctx.enter_context(tc.tile_pool(name="acts", bufs=1))
    smallp = ctx.enter_context(tc.tile_pool(name="small", bufs=2))
    dvep = ctx.enter_context(tc.tile_pool(name="dvework", bufs=3))
    h1p = ctx.enter_context(tc.tile_pool(name="h1p", bufs=2))

    # PSUM budget (8 banks): "feat"x2 + "bp"x2 + left0..left3 x1
    ps_a = ctx.enter_context(tc.tile_pool(name="psa", bufs=2, space="PSUM"))
    ps_b = ctx.enter_context(tc.tile_pool(name="psb", bufs=2, space="PSUM"))
    ps_c = ctx.enter_context(tc.tile_pool(name="psc", bufs=1, space="PSUM"))

    # ---------------- constants (small, issue before the big adj DMAs) ----
    def ctile(dram, shape, dt):
        nm = f"c_{dram.name}"
        t = cpool.tile(list(shape), dt, tag=nm, name=nm)
        nc.sync.dma_start(out=t[:], in_=dram[:])
        return t

    pw1 = ctile(d["pw1p"], [67, 25], BF16)
    pw2 = ctile(d["pw2d"], [25, 50], BF16)
    w3t4 = ctile(d["w3t4d"], [114, 128], BF16)
    gws = [None, ctile(d["gw1d"], [128, 128], BF16),
           ctile(d["gw2d"], [128, 128], BF16),
           ctile(d["gw3d"], [128, 50], BF16)]
    aw1a = ctile(d["aw1ad"], [50, 128], F32)
    aw1b = ctile(d["aw1bd"], [50, 72], F32)
    aw2a = ctile(d["aw2ad"], [128, 100], F32)
    aw2b = ctile(d["aw2bd"], [72, 100], F32)
    aw3 = ctile(d["aw3d"], [100, 100], F32)
    gw0L = ctile(d["gw0Ld"], [100, 128], F32)
    pb3fr = ctile(d["pb3frd"], [1, 128], BF16)
    selfA = ctile(d["selfAd"], [14, 128], BF16)
    selfB = ctile(d["selfBd"], [14, 62], BF16)
    biases = ctile(biasd, [128, 12], F32)
    maskT = ctile(maskTd, [50, BC], F32)
    ones1 = cpool.tile([1, 128], BF16, tag="ones1", name="ones1")
    nc.vector.memset(ones1[:], 1.0)

    def bcol(col, p0, p1):
        return biases[p0:p1, col:col + 1]

    # ---------------- resident adj^T (fp8, full) ----------------
    adjbig = resp.tile([128, JT * N], F8, tag="adj", name="adjbig")
    adj3 = adjbig[:].rearrange("p (q n) -> p q n", n=N)
    adjr = d["adjT8"][:].rearrange("(q p) c -> p q c", p=128)  # [128,32,4096]
    for q0 in range(0, JT, 4):
        nc.sync.dma_start(out=adj3[:, q0:q0 + 4, :], in_=adjr[:, q0:q0 + 4, :])

    # ---------------- activation tiles ----------------
    xt = actp.tile([128, BC * N], BF16, tag="x")          # [feat, b*N+n]
    # fcst padding columns 42:64 / 114:128 only feed PSUM partitions that
    # are never read back, but keep them zeroed: zero stationary columns
    # toggle less PE logic than garbage bits.
    fcst = actp.tile([128, JT * 128], F8, tag="fcst")     # node-major f_cut
    nc.scalar.memzero(fcst[:])
    fc3 = fcst[:].rearrange("p (q n) -> p q n", n=128)    # [128, 32, 128]
    # h2o: rows 0:4 one-hot, rows 4:64 zeros (alignment padding),
    # rows 64:114 h2 — one contraction for the layer-0 feature matmuls
    h2o = [actp.tile([114, N], BF16, tag=f"h2o{b}", name=f"h2o{b}")
           for b in range(BC)]
    nc.gpsimd.dma_start(out=h2o[0][0:64, :], in_=onehotd[0, :, :])
    nc.scalar.dma_start(out=h2o[1][0:64, :], in_=onehotd[1, :, :])
    cvec = actp.tile([128, BC], F32, tag="cvec")
    cvb = [actp.tile([1, 128], BF16, tag=f"cvb{b}", name=f"cvb{b}")
           for b in range(BC)]
    mx = actp.tile([114, NB], F32, tag="mx")
    outsb = actp.tile([114, 1], F32, tag="outsb")

    # ---------------- action MLP (tiny, fp32) ----------------
    pa = ps_a.tile([128, 2], F32, tag="feat")
    nc.tensor.matmul(pa[:], lhsT=aw1a[:], rhs=maskT[:], start=True, stop=True)
    a1a = smallp.tile([128, 2], F32, tag="a1a")
    nc.scalar.activation(a1a[:], pa[:], AF.Relu, bias=bcol(0, 0, 128))
    pb = ps_a.tile([72, 2], F32, tag="feat")
    nc.tensor.matmul(pb[:], lhsT=aw1b[:], rhs=maskT[:], start=True, stop=True)
    a1b = smallp.tile([72, 2], F32, tag="a1b")
    nc.scalar.activation(a1b[:], pb[:], AF.Relu, bias=bcol(1, 0, 72))
    pc = ps_a.tile([100, 2], F32, tag="feat")
    nc.tensor.matmul(pc[:], lhsT=aw2a[:], rhs=a1a[:], start=True, stop=False)
    nc.tensor.matmul(pc[:], lhsT=aw2b[:], rhs=a1b[:], start=False, stop=True)
    a2 = smallp.tile([100, 2], F32, tag="a2")
    nc.scalar.activation(a2[:], pc[:], AF.Relu, bias=bcol(2, 0, 100))
    pd = ps_a.tile([100, 2], F32, tag="feat")
    nc.tensor.matmul(pd[:], lhsT=aw3[:], rhs=a2[:], start=True, stop=True)
    a3 = smallp.tile([100, 2], F32, tag="a3")
    nc.scalar.activation(a3[:], pd[:], AF.Identity, bias=bcol(3, 0, 100))
    # cvec[f, b] = (a3 @ gw0[:100,:]) + pb3f  (bias for layer-0 features)
    pe_ = ps_a.tile([128, 2], F32, tag="feat")
    nc.tensor.matmul(pe_[:], lhsT=gw0L[:], rhs=a3[:], start=True, stop=True)
    nc.scalar.activation(cvec[:], pe_[:], AF.Identity, bias=bcol(4, 0, 128))
    # row versions cvb[b] = cvec[:, b].T for the node-major layer-0 matmul
    for b in range(BC):
        pr = ps_b.tile([1, 128], F32, tag="bp")
        nc.tensor.matmul(pr[:], lhsT=a3[:, b:b + 1], rhs=gw0L[:],
                         start=True, stop=False)
        nc.tensor.matmul(pr[:], lhsT=ones1[0:1, 0:1], rhs=pb3fr[:],
                         start=False, stop=True)
        nc.vector.tensor_copy(cvb[b][:], pr[:])
    # w3t4c[b] = w3t4 with the one-hot rows bumped by cvb[b]: folds the
    # action-embedding bias into the node-major layer-0 matmul
    w3t4c = [actp.tile([114, 128], BF16, tag=f"w3t4c{b}", name=f"w3t4c{b}")
             for b in range(BC)]
    for b in range(BC):
        pt4 = ps_b.tile([4, 128], F32, tag="bp")
        nc.tensor.matmul(pt4[:], lhsT=ones1[0:1, 0:4], rhs=cvb[b][0:1, :],
                         start=True, stop=True)
        nc.vector.tensor_copy(w3t4c[b][:], w3t4[:])
        nc.vector.tensor_add(w3t4c[b][0:4, :], w3t4[0:4, :], pt4[:])

    # ---------------- GCN layers ----------------
    def phase_bp(li, jt0, jt1):
        """Produce node-major fcst (fp8, x FSCALE): fc3[p, jt, 64b+c]."""
        cd = 42 if li < 3 else 50
        grp = 504 // cd  # jt groups per PSUM bank
        for b in range(BC):
            jt = jt0
            while jt < jt1:
                ng = min(grp, jt1 - jt)
                pg = ps_b.tile([128, grp * cd], F32, tag="bp")
                pg3 = pg[:].rearrange("p (g c) -> p g c", c=cd)
                for g in range(ng):
                    jc = slice((jt + g) * 128, (jt + g) * 128 + 128)
                    if li == 0:
                        nc.tensor.matmul(pg3[:, g, :], lhsT=h2o[b][:, jc],
                                         rhs=w3t4c[b][:, 0:cd],
                                         start=True, stop=True)
                    else:
                        nc.tensor.matmul(pg3[:, g, :],
                                         lhsT=xt[:, b * N + (jt + g) * 128:
                                                 b * N + (jt + g) * 128 + 128],
                                         rhs=gws[li][:, 0:cd],
                                         start=True, stop=True)
                dst = fc3[:, jt:jt + ng, 64 * b:64 * b + cd]
                src = pg3[:, 0:ng, :]
                if b == 0:
                    nc.vector.tensor_scalar_mul(dst, src, FSCALE)
                else:
                    nc.scalar.activation(dst, src, AF.Identity, scale=FSCALE)
                jt += ng

    def phase_a(li):
        """Feature-major right part: xt[32:128] = relu(f[32:128])."""
        for b in range(BC):
            for ch in range(NB):
                xs = slice(b * N + ch * 512, b * N + (ch + 1) * 512)
                pf = ps_a.tile([128, 512], F32, tag="feat")
                nc.tensor.matmul(pf[:], lhsT=gws[li][:],
                                 rhs=xt[:, xs], start=True, stop=True)
                nc.vector.tensor_scalar_max(xt[32:64, xs], pf[32:64, :],
                                            0.0)
                nc.scalar.activation(xt[64:128, xs], pf[64:128, :],
                                     AF.Relu)

    def c_alloc(half):
        return {ib: ps_c.tile([114, 512], F32, tag=f"left{ib % 4}",
                              name=f"left{ib % 4}") for ib in half}

    def c_half(li, pls, t0, t1):
        cd = 42 if li < 3 else 50
        W = 64 + cd
        for t in range(t0, t1):
            for ib in pls:
                nc.tensor.matmul(
                    pls[ib][0:W, :],
                    lhsT=fc3[:, 2 * t:2 * t + 2, 0:W],
                    rhs=adj3[:, 2 * t:2 * t + 2,
                             ib * 512:(ib + 1) * 512],
                    start=(t == 0), stop=(t == JT // 2 - 1),
                    perf_mode=DR)

    def c_acts(li, pls):
        if True:
            for ib in pls:
                if li < 3:
                    # xt[0:42] keeps the 2^16 (ASCALE*FSCALE) scale; the
                    # next layer's gw rows 0:42 are descaled on the host and
                    # the gb biases prescaled, so relu needs no scale here.
                    xs0 = slice(ib * 512, (ib + 1) * 512)
                    xs1 = slice(N + ib * 512, N + (ib + 1) * 512)
                    nc.vector.tensor_scalar(
                        xt[0:42, xs0], pls[ib][0:42, :],
                        bcol(7 + li, 0, 42), 0.0, ADD, MAX)
                    nc.scalar.activation(
                        xt[0:42, xs1], pls[ib][64:106, :], AF.Relu,
                        bias=bcol(7 + li, 64, 106))
                else:
                    # one 114-partition reduce; rows 50:64 are garbage but
                    # never read back on the host
                    nc.vector.tensor_reduce(
                        mx[:, ib:ib + 1], pls[ib][:, :],
                        mybir.AxisListType.X, mybir.AluOpType.max)


    # ---------------- positional front-end (+ layer-0 A/B\' interleave) ---
    # m6 rows: 0:6 mesh hi (b,c), 6:12 mesh lo, 12 = 0.25, 13 = 1.0 (all
    # host-built, one DMA).  psA rows 0:62 = t, rows 64:126 = t + MAGIC (the
    # PE accumulates the row-13 MAGIC term last, rounding t to the nearest
    # integer in fp32).  psB = (t + 0.25) + MAGIC.
    ADD, MAX = mybir.AluOpType.add, mybir.AluOpType.max
    peins = []
    for i in range(4):
        pt = cpool.tile([67, 512], BF16, tag=f"pein{i}", name=f"pein{i}")
        nc.vector.memset(pt[:], 0.0)
        peins.append(pt)
    m6s = [cpool.tile([14, 512], BF16, tag=f"m6_{i}", name=f"m6_{i}")
           for i in range(3)]
    # prefetch the first two mesh chunks before entering the loop
    for k in range(2):
        nc.gpsimd.dma_start(out=m6s[k][:],
                            in_=meshHLd[:, k * 512:(k + 1) * 512])

    def a0_chunk(b, ch):
        cs = slice(ch * 512, (ch + 1) * 512)
        xs = slice(b * N + ch * 512, b * N + (ch + 1) * 512)
        pf = ps_a.tile([128, 512], F32, tag="feat")
        nc.tensor.matmul(pf[:], lhsT=w3t4[:], rhs=h2o[b][:, cs],
                         start=True, stop=True)
        nc.vector.tensor_scalar(xt[32:64, xs], pf[32:64, :],
                                cvec[32:64, b:b + 1], 0.0, ADD, MAX)
        nc.scalar.activation(xt[64:128, xs], pf[64:128, :],
                             AF.Relu, bias=cvec[64:128, b:b + 1])

    pls0 = c_alloc(range(0, 4))
    for ch in range(NB):
        cs = slice(ch * 512, (ch + 1) * 512)
        m6 = m6s[ch % 3]
        if ch + 2 < NB:
            nc.gpsimd.dma_start(
                out=m6s[(ch + 2) % 3][:],
                in_=meshHLd[:, (ch + 2) * 512:(ch + 3) * 512])
        psA = ps_b.tile([128, 512], F32, tag="bp")
        nc.tensor.matmul(psA[:], lhsT=selfA[:], rhs=m6[:], start=True,
                         stop=True)
        psB = ps_b.tile([62, 512], F32, tag="bp")
        nc.tensor.matmul(psB[:], lhsT=selfB[:], rhs=m6[:], start=True,
                         stop=True)
        # rr rows 0:62 = round(t); rows 64:126 = round(t+0.25) (exact:
        # Sterbenz cancellation of MAGIC)
        rr = dvep.tile([128, 512], F32, tag="rr")
        nc.vector.tensor_scalar_add(rr[0:62, :], psA[64:126, :], -MAGIC)
        # rows 64:126 = round(t+0.25) - 0.25 (both subtractions exact)
        nc.vector.tensor_scalar(rr[64:126, :], psB[:], -MAGIC, -0.25,
                                ADD, ADD)
        # ddc rows 0:62 = t - round(t); rows 64:126 = (t+.25) - round(t+.25)
        ddc = dvep.tile([128, 512], F32, tag="ddc")
        nc.vector.tensor_sub(ddc[0:62, :], psA[0:62, :], rr[0:62, :])
        nc.vector.tensor_sub(ddc[64:126, :], psA[0:62, :], rr[64:126, :])
        for b in range(BC):
            pein = peins[2 * b + ch % 2]
            nc.scalar.activation(pein[0:30, :], ddc[32 * b:32 * b + 30, :],
                                 AF.Sin, scale=TWO_PI)
            nc.scalar.activation(pein[32:62, :],
                                 ddc[64 + 32 * b:64 + 32 * b + 30, :],
                                 AF.Sin, scale=TWO_PI)
            nc.gpsimd.dma_start(out=pein[64:67, :],
                                in_=meshHLd[3 * b:3 * b + 3, cs])
            # h1 = relu(pe_in @ pw1 + pb1)
            ph1 = ps_a.tile([25, 512], F32, tag="feat")
            nc.tensor.matmul(ph1[:], lhsT=pw1[:], rhs=pein[:],
                             start=True, stop=True)
            h1t = h1p.tile([25, 512], BF16, tag="h1")
            nc.scalar.activation(h1t[:], ph1[:], AF.Relu, bias=bcol(5, 0, 25))
            # h2 = relu(h1 @ pw2 + pb2)  (relu+bias on DVE to unload scalar)
            ph2 = ps_a.tile([50, 512], F32, tag="feat")
            nc.tensor.matmul(ph2[:], lhsT=pw2[:], rhs=h1t[:],
                             start=True, stop=True)
            nc.vector.tensor_scalar(h2o[b][64:114, cs], ph2[:],
                                    bcol(6, 0, 50), 0.0, ADD, MAX)
        for b in range(BC):
            a0_chunk(b, ch)
        if ch == 2:
            phase_bp(0, 0, 12)
        elif ch == 5:
            phase_bp(0, 12, 24)
            c_half(0, pls0, 0, 4)
        elif ch == 7:
            phase_bp(0, 24, JT)
            c_half(0, pls0, 4, 10)

    c_half(0, pls0, 10, JT // 2)
    c_acts(0, pls0)
    plsB = c_alloc(range(4, NB))
    c_half(0, plsB, 0, JT // 2)
    c_acts(0, plsB)
    for li in (1, 2, 3):
        if li < 3:
            phase_a(li)
        phase_bp(li, 0, JT)
        pls = c_alloc(range(0, 4))
        c_half(li, pls, 0, JT // 2)
        c_acts(li, pls)
        pls = c_alloc(range(4, NB))
        c_half(li, pls, 0, JT // 2)
        c_acts(li, pls)

    # ---------------- final max + bias + output ----------------
    mxr = smallp.tile([114, 1], F32, tag="mxr")
    nc.vector.tensor_reduce(mxr[:], mx[:], mybir.AxisListType.X,
                            mybir.AluOpType.max)
    nc.scalar.activation(outsb[:], mxr[:], AF.Identity, bias=bcol(10, 0, 114),
                         scale=OSCALE)
    nc.sync.dma_start(out=outd[:], in_=outsb[:])
    ctx.close()


# ---------------------------------------------------------------------------
# host side
# ---------------------------------------------------------------------------

def _descale_gw(gw):
    """Rows 0:42 consume the 2^16-scaled adjacency output; descale (exact)."""
    g = gw.astype(np.float32).copy()
    g[0:42] *= np.float32(1.0 / (ASCALE * FSCALE))
    return g.astype(BF)


def _prep_shared(inp):
    """Host preprocessing shared across cores (weights + adj)."""
    f32 = np.float32
    adjT8 = (np.ascontiguousarray(inp["adj"].astype(f32).T)
             * np.float32(ASCALE)).astype(E4)

    gw0 = inp["gw0"].astype(f32)
    w3fold = (inp["pw3"].astype(f32) @ gw0[100:200]).astype(BF)
    t4 = (inp["emb"].astype(f32) @ gw0[200:300]).astype(BF)
    pb3f = (inp["pb3"].astype(f32) @ gw0[100:200]).astype(f32)
    w3t4 = np.zeros((114, 128), BF)
    w3t4[0:4] = t4
    w3t4[64:114] = w3fold

    # pe_in row permutation: ours = [sin(f,c) x30 | cos(f,c) x30 | mesh x3]
    pw1f = inp["pw1"].astype(f32)
    pw1p_ = np.zeros((67, 25), f32)
    for k in range(30):
        f, c = divmod(k, 3)
        pw1p_[k] = pw1f[f * 6 + c]          # sin rows
        pw1p_[32 + k] = pw1f[f * 6 + 3 + c]  # cos rows
    pw1p_[64:67] = pw1f[60:63]
    pw1p = pw1p_.astype(BF)

    freqs = np.asarray([np.pi] + [2.0 * np.pi * i for i in range(1, 10)], f32)
    freq2 = np.repeat(freqs, 3) / (2.0 * np.pi)   # [30]
    self6 = np.zeros((6, 62), f32)
    for b in range(2):
        for k in range(30):
            self6[3 * b + k % 3, 32 * b + k] = freq2[k]
    # m6 rows: 0:6 mesh hi, 6:12 mesh lo, 12 = 0.25, 13 = 1.0.  selfA maps
    # them to t (cols 0:62) and t + MAGIC (cols 64:126); selfB to
    # (t + 0.25) + MAGIC.  freq2 entries and MAGIC are bf16-exact; the PE
    # accumulates rows in order, so the MAGIC term lands last and rounds
    # t (resp. t + 0.25) to the nearest integer in the fp32 accumulator.
    selfA = np.zeros((14, 128), f32)
    selfA[0:6, 0:62] = self6
    selfA[0:6, 64:126] = self6
    selfA[6:12, 0:62] = self6
    selfA[6:12, 64:126] = self6
    selfA[13, 64:126] = np.float32(MAGIC)
    selfB = np.zeros((14, 62), f32)
    selfB[0:6, :] = self6
    selfB[6:12, :] = self6
    selfB[12, :] = 1.0
    selfB[13, :] = np.float32(MAGIC)

    biasd = np.zeros((128, 12), f32)
    biasd[0:128, 0] = inp["ab1"][:128]
    biasd[0:72, 1] = inp["ab1"][128:200]
    biasd[0:100, 2] = inp["ab2"]
    biasd[0:100, 3] = inp["ab3"]
    biasd[0:128, 4] = pb3f
    biasd[0:25, 5] = inp["pb1"].astype(f32)
    biasd[0:50, 6] = inp["pb2"].astype(f32)
    # gb biases for layers 0-2 prescaled by 2^16 = ASCALE*FSCALE: the
    # post-adjacency relu output keeps that scale, and the next layer's gw
    # rows 0:42 are descaled to compensate.
    XS = np.float32(ASCALE * FSCALE)
    for li in range(3):
        biasd[0:42, 7 + li] = inp[f"gb{li}"].astype(f32)[:42] * XS
        biasd[64:106, 7 + li] = inp[f"gb{li}"].astype(f32)[:42] * XS
    biasd[0:50, 10] = inp["gb3"].astype(f32)
    biasd[64:114, 10] = inp["gb3"].astype(f32)

    return {
        "adjT8": adjT8,
        "pw1p": pw1p,
        "pw2d": inp["pw2"].astype(BF),
        "w3t4d": w3t4,
        "gw1d": _descale_gw(inp["gw1"]),
        "gw2d": _descale_gw(inp["gw2"]),
        "gw3d": _descale_gw(inp["gw3"]),
        "aw1ad": np.ascontiguousarray(inp["aw1"].astype(f32)[:, :128]),
        "aw1bd": np.ascontiguousarray(inp["aw1"].astype(f32)[:, 128:200]),
        "aw2ad": np.ascontiguousarray(inp["aw2"].astype(f32)[:128]),
        "aw2bd": np.ascontiguousarray(inp["aw2"].astype(f32)[128:200]),
        "aw3d": inp["aw3"].astype(f32),
        "gw0Ld": np.ascontiguousarray(gw0[:100]),
        "pb3frd": pb3f.reshape(1, 128).astype(BF),
        "selfAd": selfA.astype(BF),
        "selfBd": selfB.astype(BF),
        "biasd": biasd,
    }


def _prep_core(inp, shared, core):
    bs = slice(core * BC, (core + 1) * BC)
    f32 = np.float32
    mesh = inp["mesh"].astype(f32)[bs]                       # [2, N, 3]
    meshT = mesh.transpose(0, 2, 1).reshape(6, N)            # rows (b,c)
    hi = meshT.astype(BF)
    lo = (meshT - hi.astype(f32)).astype(BF)
    meshHL = np.zeros((14, N), BF)
    meshHL[0:6] = hi
    meshHL[6:12] = lo
    meshHL[12] = BF(0.25)
    meshHL[13] = BF(1.0)
    mi = inp["mask_idx"][bs]                                 # [2, N] int32
    oh = (mi[:, None, :] == np.arange(4, dtype=mi.dtype)[None, :, None])
    onehot = np.zeros((BC, 64, N), BF)
    onehot[:, 0:4, :] = oh.astype(BF)
    maskT = np.ascontiguousarray(inp["mask"].astype(f32)[bs].T)  # [50, 2]
    m = dict(shared)
    m["meshHLd"] = meshHL
    m["onehotd"] = onehot
    m["maskTd"] = maskT
    return m


_CACHED = {}


def kernel(**inputs) -> np.ndarray:
    if "nc" not in _CACHED:
        _CACHED["nc"] = build_bass()
    nc = _CACHED["nc"]
    shared = _prep_shared(inputs)
    in_maps = [_prep_core(inputs, shared, c) for c in range(NCORES)]
    res = run_bass_kernel_spmd(nc, in_maps, list(range(NCORES)), **run_kwargs)
    out = np.empty((B, 50), np.float32)
    for c in range(NCORES):
        o = res.results[c]["outd"][:, 0]
        out[2 * c] = o[0:50]
        out[2 * c + 1] = o[64:114]
    _CACHED["last_results"] = res
    return out


# revision 31
# speedup vs baseline: 1.0212x; 1.0189x over previous
"""Trainium2 Bass kernel for the GNN message-passing model.

Strategy: pure data-parallel over batch (B=16 -> 2 batches per core, 8 cores,
no cross-core communication). Activations are feature-major
([feat, batch*node]) for the per-node matmuls. The adjacency matmul keeps the
FULL adjacency SBUF-resident as fp8 e4m3 (adj^T * 4096, 16.8 MB) and runs in
DoubleRow fp8 perf mode (2 contraction rows/cycle): stationary operand is the
node-major cut-feature tile (fp8, scaled by 16), moving operand is a pair of
adj^T j-tiles. The node-major f_cut is produced directly by a second small
matmul per 128-node tile (stationary = x-slice, moving = cut columns of the
layer weight), avoiding PE transposes + PSUM casts. The 1/(4096*16) descale
is folded into the post-adjacency activation's scale.

Weight-only folds done on host (pure parameter preprocessing):
  W3fold = pw3 @ gw0[100:200]   (positional-MLP last layer folded into gw0)
  t4     = emb @ gw0[200:300]   (embedding table folded into gw0)
  pb3f   = pb3 @ gw0[100:200]   (bias fold)
mask_idx is re-encoded as a one-hot (4 classes, exact in fp8) so the
embedding lookup becomes a K=4 matmul accumulated into the same PSUM.
"""

import numpy as np
import ml_dtypes

import concourse.bass as bass
import concourse.mybir as mybir
import concourse.tile as tile
import concourse.bass_utils as _bass_utils
from concourse.bass_utils import run_bass_kernel_spmd

# (walrus's --enable-ldw-opt pass was tried and rejects this kernel's
# DoubleRow self-loading matmuls; keep the default.)

F32 = mybir.dt.float32
BF16 = mybir.dt.bfloat16
F8 = mybir.dt.float8e4
AF = mybir.ActivationFunctionType
DR = mybir.MatmulPerfMode.DoubleRow
BF = ml_dtypes.bfloat16
E4 = ml_dtypes.float8_e4m3

B, N, BC = 16, 4096, 2          # batches, nodes, batches per core
NCORES = 8
NB = N // 512                   # 8 i-blocks of 512 output nodes
JT = N // 128                   # 32 contraction j-tiles
MAGIC = float(1.5 * 2 ** 23)    # fp32 round-to-nearest magic constant
TWO_PI = float(2.0 * np.pi)
ASCALE = 4096.0                 # adj premultiplier (host, before fp8 cast)
FSCALE = 16.0                   # f_cut premultiplier (device, before fp8)
OSCALE = float(1.0 / (ASCALE * FSCALE))

run_kwargs = {}                 # test.py may inject trace kwargs here


def split_excess_waits(nc, max_waits=1):
    """Walrus codegen on this image rejects >1 sem wait per instruction;
    move excess waits onto preceding same-engine no-ops."""
    n_split = 0
    for fn in nc.m.functions:
        for blk in fn.blocks:
            insts = list(blk.instructions)
            out = []
            changed = False
            for inst in insts:
                si = getattr(inst, "sync_info", None)
                if si is not None and len(si.on_wait) > max_waits:
                    waits = list(si.on_wait)
                    chunks = [waits[i:i + max_waits]
                              for i in range(0, len(waits), max_waits)]
                    for ci, ch in enumerate(chunks[:-1]):
                        nop = mybir.InstNoOp(
                            name=f"{inst.name}-wsplit-{ci}", ins=[], outs=[])
                        nop.engine = inst.engine
                        nop.sync_info = mybir.SyncInfo(on_wait=ch, on_update=[])
                        out.append(nop)
                        n_split += 1
                    inst.sync_info = mybir.SyncInfo(
                        on_wait=chunks[-1], on_update=list(si.on_update))
                    changed = True
                out.append(inst)
            if changed:
                blk.instructions = out
    return n_split


def _param(nc, name, shape, dt):
    return nc.declare_dram_parameter(name, list(shape), dt, isOutput=False)


def build_bass(split=True):
    nc = bass.Bass()

    adjT8 = _param(nc, "adjT8", [N, N], F8)
    meshHLd = _param(nc, "meshHLd", [14, N], BF16)
    onehotd = _param(nc, "onehotd", [BC, 64, N], BF16)
    maskTd = _param(nc, "maskTd", [50, BC], F32)

    pw1p = _param(nc, "pw1p", [67, 25], BF16)
    pw2d = _param(nc, "pw2d", [25, 50], BF16)
    w3t4d = _param(nc, "w3t4d", [114, 128], BF16)
    gw1d = _param(nc, "gw1d", [128, 128], BF16)
    gw2d = _param(nc, "gw2d", [128, 128], BF16)
    gw3d = _param(nc, "gw3d", [128, 50], BF16)
    aw1ad = _param(nc, "aw1ad", [50, 128], F32)
    aw1bd = _param(nc, "aw1bd", [50, 72], F32)
    aw2ad = _param(nc, "aw2ad", [128, 100], F32)
    aw2bd = _param(nc, "aw2bd", [72, 100], F32)
    aw3d = _param(nc, "aw3d", [100, 100], F32)
    gw0Ld = _param(nc, "gw0Ld", [100, 128], F32)
    pb3frd = _param(nc, "pb3frd", [1, 128], BF16)
    selfAd = _param(nc, "selfAd", [14, 128], BF16)
    selfBd = _param(nc, "selfBd", [14, 62], BF16)
    biasd = _param(nc, "biasd", [128, 12], F32)
    # bias columns (within biasd): 0 ab1a[128], 1 ab1b[72], 2 ab2[100],
    # 3 ab3[100], 4 pb3f[128], 5 pb1[25], 6 pb2[50], 7 gbl0[x2@0/64],
    # 8 gbl1, 9 gbl2, 10 gb3x2[114]
    outd = nc.declare_dram_parameter("outd", [114, 1], F32, isOutput=True)

    with tile.TileContext(nc) as tc:
        _emit(nc, tc, locals())
    if split:
        split_excess_waits(nc)
    return nc


def _emit(nc, tc, d):
    import contextlib
    ctx = contextlib.ExitStack()
    meshHLd, onehotd, maskTd = d["meshHLd"], d["onehotd"], d["maskTd"]
    biasd, outd = d["biasd"], d["outd"]

    cpool = ctx.enter_context(tc.tile_pool(name="consts", bufs=1))
    resp = ctx.enter_context(tc.tile_pool(name="resadj", bufs=1))
    actp = ctx.enter_context(tc.tile_pool(name="acts", bufs=1))
    smallp = ctx.enter_context(tc.tile_pool(name="small", bufs=2))
    dvep = ctx.enter_context(tc.tile_pool(name="dvework", bufs=3))
    h1p = ctx.enter_context(tc.tile_pool(name="h1p", bufs=2))

    # PSUM budget (8 banks): "feat"x2 + "bp"x2 + left0..left3 x1
    ps_a = ctx.enter_context(tc.tile_pool(name="psa", bufs=2, space="PSUM"))
    ps_b = ctx.enter_context(tc.tile_pool(name="psb", bufs=2, space="PSUM"))
    ps_c = ctx.enter_context(tc.tile_pool(name="psc", bufs=1, space="PSUM"))

    # ---------------- constants (small, issue before the big adj DMAs) ----
    def ctile(dram, shape, dt):
        nm = f"c_{dram.name}"
        t = cpool.tile(list(shape), dt, tag=nm, name=nm)
        nc.sync.dma_start(out=t[:], in_=dram[:])
        return t

    pw1 = ctile(d["pw1p"], [67, 25], BF16)
    pw2 = ctile(d["pw2d"], [25, 50], BF16)
    w3t4 = ctile(d["w3t4d"], [114, 128], BF16)
    gws = [None, ctile(d["gw1d"], [128, 128], BF16),
           ctile(d["gw2d"], [128, 128], BF16),
           ctile(d["gw3d"], [128, 50], BF16)]
    aw1a = ctile(d["aw1ad"], [50, 128], F32)
    aw1b = ctile(d["aw1bd"], [50, 72], F32)
    aw2a = ctile(d["aw2ad"], [128, 100], F32)
    aw2b = ctile(d["aw2bd"], [72, 100], F32)
    aw3 = ctile(d["aw3d"], [100, 100], F32)
    gw0L = ctile(d["gw0Ld"], [100, 128], F32)
    pb3fr = ctile(d["pb3frd"], [1, 128], BF16)
    selfA = ctile(d["selfAd"], [14, 128], BF16)
    selfB = ctile(d["selfBd"], [14, 62], BF16)
    biases = ctile(biasd, [128, 12], F32)
    maskT = ctile(maskTd, [50, BC], F32)
    ones1 = cpool.tile([1, 128], BF16, tag="ones1", name="ones1")
    nc.vector.memset(ones1[:], 1.0)

    def bcol(col, p0, p1):
        return biases[p0:p1, col:col + 1]

    # ---------------- resident adj^T (fp8, full) ----------------
    adjbig = resp.tile([128, JT * N], F8, tag="adj", name="adjbig")
    adj3 = adjbig[:].rearrange("p (q n) -> p q n", n=N)
    adjr = d["adjT8"][:].rearrange("(q p) c -> p q c", p=128)  # [128,32,4096]
    for q0 in range(0, JT, 4):
        nc.sync.dma_start(out=adj3[:, q0:q0 + 4, :], in_=adjr[:, q0:q0 + 4, :])

    # ---------------- activation tiles ----------------
    xt = actp.tile([128, BC * N], BF16, tag="x")          # [feat, b*N+n]
    # fcst padding columns 42:64 / 114:128 only feed PSUM partitions that
    # are never read back, but keep them zeroed: zero stationary columns
    # toggle less PE logic than garbage bits.
    fcst = actp.tile([128, JT * 128], F8, tag="fcst")     # node-major f_cut
    nc.scalar.memzero(fcst[:])
    fc3 = fcst[:].rearrange("p (q n) -> p q n", n=128)    # [128, 32, 128]
    # h2o: rows 0:4 one-hot, rows 4:64 zeros (alignment padding),
    # rows 64:114 h2 — one contraction for the layer-0 feature matmuls
    h2o = [actp.tile([114, N], BF16, tag=f"h2o{b}", name=f"h2o{b}")
           for b in range(BC)]
    nc.gpsimd.dma_start(out=h2o[0][0:64, :], in_=onehotd[0, :, :])
    nc.scalar.dma_start(out=h2o[1][0:64, :], in_=onehotd[1, :, :])
    cvec = actp.tile([128, BC], F32, tag="cvec")
    cvb = [actp.tile([1, 128], BF16, tag=f"cvb{b}", name=f"cvb{b}")
           for b in range(BC)]
    mx = actp.tile([114, NB], F32, tag="mx")
    outsb = actp.tile([114, 1], F32, tag="outsb")

    # ---------------- action MLP (tiny, fp32) ----------------
    pa = ps_a.tile([128, 2], F32, tag="feat")
    nc.tensor.matmul(pa[:], lhsT=aw1a[:], rhs=maskT[:], start=True, stop=True)
    a1a = smallp.tile([128, 2], F32, tag="a1a")
    nc.scalar.activation(a1a[:], pa[:], AF.Relu, bias=bcol(0, 0, 128))
    pb = ps_a.tile([72, 2], F32, tag="feat")
    nc.tensor.matmul(pb[:], lhsT=aw1b[:], rhs=maskT[:], start=True, stop=True)
    a1b = smallp.tile([72, 2], F32, tag="a1b")
    nc.scalar.activation(a1b[:], pb[:], AF.Relu, bias=bcol(1, 0, 72))
    pc = ps_a.tile([100, 2], F32, tag="feat")
    nc.tensor.matmul(pc[:], lhsT=aw2a[:], rhs=a1a[:], start=True, stop=False)
    nc.tensor.matmul(pc[:], lhsT=aw2b[:], rhs=a1b[:], start=False, stop=True)
    a2 = smallp.tile([100, 2], F32, tag="a2")
    nc.scalar.activation(a2[:], pc[:], AF.Relu, bias=bcol(2, 0, 100))
    pd = ps_a.tile([100, 2], F32, tag="feat")
    nc.tensor.matmul(pd[:], lhsT=aw3[:], rhs=a2[:], start=True, stop=True)
    a3 = smallp.tile([100, 2], F32, tag="a3")
    nc.scalar.activation(a3[:], pd[:], AF.Identity, bias=bcol(3, 0, 100))
    # cvec[f, b] = (a3 @ gw0[:100,:]) + pb3f  (bias for layer-0 features)
    pe_ = ps_a.tile([128, 2], F32, tag="feat")
    nc.tensor.matmul(pe_[:], lhsT=gw0L[:], rhs=a3[:], start=True, stop=True)
    nc.scalar.activation(cvec[:], pe_[:], AF.Identity, bias=bcol(4, 0, 128))
    # row versions cvb[b] = cvec[:, b].T for the node-major layer-0 matmul
    for b in range(BC):
        pr = ps_b.tile([1, 128], F32, tag="bp")
        nc.tensor.matmul(pr[:], lhsT=a3[:, b:b + 1], rhs=gw0L[:],
                         start=True, stop=False)
        nc.tensor.matmul(pr[:], lhsT=ones1[0:1, 0:1], rhs=pb3fr[:],
                         start=False, stop=True)
        nc.vector.tensor_copy(cvb[b][:], pr[:])
    # w3t4c[b] = w3t4 with the one-hot rows bumped by cvb[b]: folds the
    # action-embedding bias into the node-major layer-0 matmul
    w3t4c = [actp.tile([114, 128], BF16, tag=f"w3t4c{b}", name=f"w3t4c{b}")
             for b in range(BC)]
    for b in range(BC):
        pt4 = ps_b.tile([4, 128], F32, tag="bp")
        nc.tensor.matmul(pt4[:], lhsT=ones1[0:1, 0:4], rhs=cvb[b][0:1, :],
                         start=True, stop=True)
        nc.vector.tensor_copy(w3t4c[b][:], w3t4[:])
        nc.vector.tensor_add(w3t4c[b][0:4, :], w3t4[0:4, :], pt4[:])

    # ---------------- GCN layers ----------------
    def phase_bp(li, jt0, jt1):
        """Produce node-major fcst (fp8, x FSCALE): fc3[p, jt, 64b+c]."""
        cd = 42 if li < 3 else 50
        grp = 504 // cd  # jt groups per PSUM bank
        for b in range(BC):
            jt = jt0
            while jt < jt1:
                ng = min(grp, jt1 - jt)
                pg = ps_b.tile([128, grp * cd], F32, tag="bp")
                pg3 = pg[:].rearrange("p (g c) -> p g c", c=cd)
                for g in range(ng):
                    jc = slice((jt + g) * 128, (jt + g) * 128 + 128)
                    if li == 0:
                        nc.tensor.matmul(pg3[:, g, :], lhsT=h2o[b][:, jc],
                                         rhs=w3t4c[b][:, 0:cd],
                                         start=True, stop=True)
                    else:
                        nc.tensor.matmul(pg3[:, g, :],
                                         lhsT=xt[:, b * N + (jt + g) * 128:
                                                 b * N + (jt + g) * 128 + 128],
                                         rhs=gws[li][:, 0:cd],
                                         start=True, stop=True)
                dst = fc3[:, jt:jt + ng, 64 * b:64 * b + cd]
                src = pg3[:, 0:ng, :]
                if b == 0:
                    nc.vector.tensor_scalar_mul(dst, src, FSCALE)
                else:
                    nc.scalar.activation(dst, src, AF.Identity, scale=FSCALE)
                jt += ng

    def phase_a(li):
        """Feature-major right part: xt[32:128] = relu(f[32:128])."""
        for b in range(BC):
            for ch in range(NB):
                xs = slice(b * N + ch * 512, b * N + (ch + 1) * 512)
                pf = ps_a.tile([128, 512], F32, tag="feat")
                nc.tensor.matmul(pf[:], lhsT=gws[li][:],
                                 rhs=xt[:, xs], start=True, stop=True)
                nc.vector.tensor_scalar_max(xt[32:64, xs], pf[32:64, :],
                                            0.0)
                nc.scalar.activation(xt[64:128, xs], pf[64:128, :],
                                     AF.Relu)

    def c_alloc(half):
        return {ib: ps_c.tile([114, 512], F32, tag=f"left{ib % 4}",
                              name=f"left{ib % 4}") for ib in half}

    def c_half(li, pls, t0, t1):
        cd = 42 if li < 3 else 50
        W = 64 + cd
        for t in range(t0, t1):
            for ib in pls:
                nc.tensor.matmul(
                    pls[ib][0:W, :],
                    lhsT=fc3[:, 2 * t:2 * t + 2, 0:W],
                    rhs=adj3[:, 2 * t:2 * t + 2,
                             ib * 512:(ib + 1) * 512],
                    start=(t == 0), stop=(t == JT // 2 - 1),
                    perf_mode=DR)

    def c_acts(li, pls):
        if True:
            for ib in pls:
                if li < 3:
                    # xt[0:42] keeps the 2^16 (ASCALE*FSCALE) scale; the
                    # next layer's gw rows 0:42 are descaled on the host and
                    # the gb biases prescaled, so relu needs no scale here.
                    xs0 = slice(ib * 512, (ib + 1) * 512)
                    xs1 = slice(N + ib * 512, N + (ib + 1) * 512)
                    nc.vector.tensor_scalar(
                        xt[0:42, xs0], pls[ib][0:42, :],
                        bcol(7 + li, 0, 42), 0.0, ADD, MAX)
                    nc.scalar.activation(
                        xt[0:42, xs1], pls[ib][64:106, :], AF.Relu,
                        bias=bcol(7 + li, 64, 106))
                else:
                    # one 114-partition reduce; rows 50:64 are garbage but
                    # never read back on the host
                    nc.vector.tensor_reduce(
                        mx[:, ib:ib + 1], pls[ib][:, :],
                        mybir.AxisListType.X, mybir.AluOpType.max)


    # ---------------- positional front-end (+ layer-0 A/B\' interleave) ---
    # m6 rows: 0:6 mesh hi (b,c), 6:12 mesh lo, 12 = 0.25, 13 = 1.0 (all
    # host-built, one DMA).  psA rows 0:62 = t, rows 64:126 = t + MAGIC (the
    # PE accumulates the row-13 MAGIC term last, rounding t to the nearest
    # integer in fp32).  psB = (t + 0.25) + MAGIC.
    ADD, MAX = mybir.AluOpType.add, mybir.AluOpType.max
    peins = []
    for i in range(4):
        pt = cpool.tile([67, 512], BF16, tag=f"pein{i}", name=f"pein{i}")
        nc.vector.memset(pt[:], 0.0)
        peins.append(pt)
    m6s = [cpool.tile([14, 512], BF16, tag=f"m6_{i}", name=f"m6_{i}")
           for i in range(3)]
    # prefetch the first two mesh chunks before entering the loop
    for k in range(2):
        nc.gpsimd.dma_start(out=m6s[k][:],
                            in_=meshHLd[:, k * 512:(k + 1) * 512])

    def a0_chunk(b, ch):
        cs = slice(ch * 512, (ch + 1) * 512)
        xs = slice(b * N + ch * 512, b * N + (ch + 1) * 512)
        # borrow the (idle during phase 1) phase-C banks so the layer-0
        # feature matmuls don't serialize against the ph1/ph2 rotation
        pf = ps_c.tile([128, 512], F32, tag=f"left{b}", name=f"left{b}")
        nc.tensor.matmul(pf[:], lhsT=w3t4[:], rhs=h2o[b][:, cs],
                         start=True, stop=True)
        nc.vector.tensor_scalar(xt[32:64, xs], pf[32:64, :],
                                cvec[32:64, b:b + 1], 0.0, ADD, MAX)
        nc.scalar.activation(xt[64:128, xs], pf[64:128, :],
                             AF.Relu, bias=cvec[64:128, b:b + 1])

    for ch in range(NB):
        cs = slice(ch * 512, (ch + 1) * 512)
        m6 = m6s[ch % 3]
        if ch + 2 < NB:
            nc.gpsimd.dma_start(
                out=m6s[(ch + 2) % 3][:],
                in_=meshHLd[:, (ch + 2) * 512:(ch + 3) * 512])
        psA = ps_b.tile([128, 512], F32, tag="bp")
        nc.tensor.matmul(psA[:], lhsT=selfA[:], rhs=m6[:], start=True,
                         stop=True)
        psB = ps_b.tile([62, 512], F32, tag="bp")
        nc.tensor.matmul(psB[:], lhsT=selfB[:], rhs=m6[:], start=True,
                         stop=True)
        # rr rows 0:62 = round(t); rows 64:126 = round(t+0.25) (exact:
        # Sterbenz cancellation of MAGIC)
        rr = dvep.tile([128, 512], F32, tag="rr")
        nc.vector.tensor_scalar_add(rr[0:62, :], psA[64:126, :], -MAGIC)
        # rows 64:126 = round(t+0.25) - 0.25 (both subtractions exact)
        nc.vector.tensor_scalar(rr[64:126, :], psB[:], -MAGIC, -0.25,
                                ADD, ADD)
        # ddc rows 0:62 = t - round(t); rows 64:126 = (t+.25) - round(t+.25)
        ddc = dvep.tile([128, 512], F32, tag="ddc")
        nc.vector.tensor_sub(ddc[0:62, :], psA[0:62, :], rr[0:62, :])
        nc.vector.tensor_sub(ddc[64:126, :], psA[0:62, :], rr[64:126, :])
        for b in range(BC):
            pein = peins[2 * b + ch % 2]
            nc.scalar.activation(pein[0:30, :], ddc[32 * b:32 * b + 30, :],
                                 AF.Sin, scale=TWO_PI)
            nc.scalar.activation(pein[32:62, :],
                                 ddc[64 + 32 * b:64 + 32 * b + 30, :],
                                 AF.Sin, scale=TWO_PI)
            nc.gpsimd.dma_start(out=pein[64:67, :],
                                in_=meshHLd[3 * b:3 * b + 3, cs])
            # h1 = relu(pe_in @ pw1 + pb1)
            ph1 = ps_a.tile([25, 512], F32, tag="feat")
            nc.tensor.matmul(ph1[:], lhsT=pw1[:], rhs=pein[:],
                             start=True, stop=True)
            h1t = h1p.tile([25, 512], BF16, tag="h1")
            nc.scalar.activation(h1t[:], ph1[:], AF.Relu, bias=bcol(5, 0, 25))
            # h2 = relu(h1 @ pw2 + pb2)  (relu+bias on DVE to unload scalar)
            ph2 = ps_a.tile([50, 512], F32, tag="feat")
            nc.tensor.matmul(ph2[:], lhsT=pw2[:], rhs=h1t[:],
                             start=True, stop=True)
            nc.vector.tensor_scalar(h2o[b][64:114, cs], ph2[:],
                                    bcol(6, 0, 50), 0.0, ADD, MAX)
        for b in range(BC):
            a0_chunk(b, ch)
        if ch == 2:
            phase_bp(0, 0, 12)
        elif ch == 5:
            phase_bp(0, 12, 24)
        elif ch == 7:
            phase_bp(0, 24, JT)

    for li in (0, 1, 2, 3):
        if 0 < li < 3:
            phase_a(li)
        if li > 0:
            phase_bp(li, 0, JT)
        pls = c_alloc(range(0, 4))
        c_half(li, pls, 0, JT // 2)
        c_acts(li, pls)
        pls = c_alloc(range(4, NB))
        c_half(li, pls, 0, JT // 2)
        c_acts(li, pls)

    # ---------------- final max + bias + output ----------------
    mxr = smallp.tile([114, 1], F32, tag="mxr")
    nc.vector.tensor_reduce(mxr[:], mx[:], mybir.AxisListType.X,
                            mybir.AluOpType.max)
    nc.scalar.activation(outsb[:], mxr[:], AF.Identity, bias=bcol(10, 0, 114),
                         scale=OSCALE)
    nc.sync.dma_start(out=outd[:], in_=outsb[:])
    ctx.close()


# ---------------------------------------------------------------------------
# host side
# ---------------------------------------------------------------------------

def _descale_gw(gw):
    """Rows 0:42 consume the 2^16-scaled adjacency output; descale (exact)."""
    g = gw.astype(np.float32).copy()
    g[0:42] *= np.float32(1.0 / (ASCALE * FSCALE))
    return g.astype(BF)


def _prep_shared(inp):
    """Host preprocessing shared across cores (weights + adj)."""
    f32 = np.float32
    adjT8 = (np.ascontiguousarray(inp["adj"].astype(f32).T)
             * np.float32(ASCALE)).astype(E4)

    gw0 = inp["gw0"].astype(f32)
    w3fold = (inp["pw3"].astype(f32) @ gw0[100:200]).astype(BF)
    t4 = (inp["emb"].astype(f32) @ gw0[200:300]).astype(BF)
    pb3f = (inp["pb3"].astype(f32) @ gw0[100:200]).astype(f32)
    w3t4 = np.zeros((114, 128), BF)
    w3t4[0:4] = t4
    w3t4[64:114] = w3fold

    # pe_in row permutation: ours = [sin(f,c) x30 | cos(f,c) x30 | mesh x3]
    pw1f = inp["pw1"].astype(f32)
    pw1p_ = np.zeros((67, 25), f32)
    for k in range(30):
        f, c = divmod(k, 3)
        pw1p_[k] = pw1f[f * 6 + c]          # sin rows
        pw1p_[32 + k] = pw1f[f * 6 + 3 + c]  # cos rows
    pw1p_[64:67] = pw1f[60:63]
    pw1p = pw1p_.astype(BF)

    freqs = np.asarray([np.pi] + [2.0 * np.pi * i for i in range(1, 10)], f32)
    freq2 = np.repeat(freqs, 3) / (2.0 * np.pi)   # [30]
    self6 = np.zeros((6, 62), f32)
    for b in range(2):
        for k in range(30):
            self6[3 * b + k % 3, 32 * b + k] = freq2[k]
    # m6 rows: 0:6 mesh hi, 6:12 mesh lo, 12 = 0.25, 13 = 1.0.  selfA maps
    # them to t (cols 0:62) and t + MAGIC (cols 64:126); selfB to
    # (t + 0.25) + MAGIC.  freq2 entries and MAGIC are bf16-exact; the PE
    # accumulates rows in order, so the MAGIC term lands last and rounds
    # t (resp. t + 0.25) to the nearest integer in the fp32 accumulator.
    selfA = np.zeros((14, 128), f32)
    selfA[0:6, 0:62] = self6
    selfA[0:6, 64:126] = self6
    selfA[6:12, 0:62] = self6
    selfA[6:12, 64:126] = self6
    selfA[13, 64:126] = np.float32(MAGIC)
    selfB = np.zeros((14, 62), f32)
    selfB[0:6, :] = self6
    selfB[6:12, :] = self6
    selfB[12, :] = 1.0
    selfB[13, :] = np.float32(MAGIC)

    biasd = np.zeros((128, 12), f32)
    biasd[0:128, 0] = inp["ab1"][:128]
    biasd[0:72, 1] = inp["ab1"][128:200]
    biasd[0:100, 2] = inp["ab2"]
    biasd[0:100, 3] = inp["ab3"]
    biasd[0:128, 4] = pb3f
    biasd[0:25, 5] = inp["pb1"].astype(f32)
    biasd[0:50, 6] = inp["pb2"].astype(f32)
    # gb biases for layers 0-2 prescaled by 2^16 = ASCALE*FSCALE: the
    # post-adjacency relu output keeps that scale, and the next layer's gw
    # rows 0:42 are descaled to compensate.
    XS = np.float32(ASCALE * FSCALE)
    for li in range(3):
        biasd[0:42, 7 + li] = inp[f"gb{li}"].astype(f32)[:42] * XS
        biasd[64:106, 7 + li] = inp[f"gb{li}"].astype(f32)[:42] * XS
    biasd[0:50, 10] = inp["gb3"].astype(f32)
    biasd[64:114, 10] = inp["gb3"].astype(f32)

    return {
        "adjT8": adjT8,
        "pw1p": pw1p,
        "pw2d": inp["pw2"].astype(BF),
        "w3t4d": w3t4,
        "gw1d": _descale_gw(inp["gw1"]),
        "gw2d": _descale_gw(inp["gw2"]),
        "gw3d": _descale_gw(inp["gw3"]),
        "aw1ad": np.ascontiguousarray(inp["aw1"].astype(f32)[:, :128]),
        "aw1bd": np.ascontiguousarray(inp["aw1"].astype(f32)[:, 128:200]),
        "aw2ad": np.ascontiguousarray(inp["aw2"].astype(f32)[:128]),
        "aw2bd": np.ascontiguousarray(inp["aw2"].astype(f32)[128:200]),
        "aw3d": inp["aw3"].astype(f32),
        "gw0Ld": np.ascontiguousarray(gw0[:100]),
        "pb3frd": pb3f.reshape(1, 128).astype(BF),
        "selfAd": selfA.astype(BF),
        "selfBd": selfB.astype(BF),
        "biasd": biasd,
    }


def _prep_core(inp, shared, core):
    bs = slice(core * BC, (core + 1) * BC)
    f32 = np.float32
    mesh = inp["mesh"].astype(f32)[bs]                       # [2, N, 3]
    meshT = mesh.transpose(0, 2, 1).reshape(6, N)            # rows (b,c)
    hi = meshT.astype(BF)
    lo = (meshT - hi.astype(f32)).astype(BF)
    meshHL = np.zeros((14, N), BF)
    meshHL[0:6] = hi
    meshHL[6:12] = lo
    meshHL[12] = BF(0.25)
    meshHL[13] = BF(1.0)
    mi = inp["mask_idx"][bs]                                 # [2, N] int32
    oh = (mi[:, None, :] == np.arange(4, dtype=mi.dtype)[None, :, None])
    onehot = np.zeros((BC, 64, N), BF)
    onehot[:, 0:4, :] = oh.astype(BF)
    maskT = np.ascontiguousarray(inp["mask"].astype(f32)[bs].T)  # [50, 2]
    m = dict(shared)
    m["meshHLd"] = meshHL
    m["onehotd"] = onehot
    m["maskTd"] = maskT
    return m


_CACHED = {}


def kernel(**inputs) -> np.ndarray:
    if "nc" not in _CACHED:
        _CACHED["nc"] = build_bass()
    nc = _CACHED["nc"]
    shared = _prep_shared(inputs)
    in_maps = [_prep_core(inputs, shared, c) for c in range(NCORES)]
    res = run_bass_kernel_spmd(nc, in_maps, list(range(NCORES)), **run_kwargs)
    out = np.empty((B, 50), np.float32)
    for c in range(NCORES):
        o = res.results[c]["outd"][:, 0]
        out[2 * c] = o[0:50]
        out[2 * c + 1] = o[64:114]
    _CACHED["last_results"] = res
    return out


# revision 33
# speedup vs baseline: 1.0525x; 1.0306x over previous
"""Trainium2 Bass kernel for the GNN message-passing model.

Strategy: pure data-parallel over batch (B=16 -> 2 batches per core, 8 cores,
no cross-core communication). Activations are feature-major
([feat, batch*node]) for the per-node matmuls. The adjacency matmul keeps the
FULL adjacency SBUF-resident as fp8 e4m3 (adj^T * 4096, 16.8 MB) and runs in
DoubleRow fp8 perf mode (2 contraction rows/cycle): stationary operand is the
node-major cut-feature tile (fp8, scaled by 16), moving operand is a pair of
adj^T j-tiles. The node-major f_cut is produced directly by a second small
matmul per 128-node tile (stationary = x-slice, moving = cut columns of the
layer weight), avoiding PE transposes + PSUM casts. The 1/(4096*16) descale
is folded into the post-adjacency activation's scale.

Weight-only folds done on host (pure parameter preprocessing):
  W3fold = pw3 @ gw0[100:200]   (positional-MLP last layer folded into gw0)
  t4     = emb @ gw0[200:300]   (embedding table folded into gw0)
  pb3f   = pb3 @ gw0[100:200]   (bias fold)
mask_idx is re-encoded as a one-hot (4 classes, exact in fp8) so the
embedding lookup becomes a K=4 matmul accumulated into the same PSUM.
"""

import numpy as np
import ml_dtypes

import concourse.bass as bass
import concourse.mybir as mybir
import concourse.tile as tile
import concourse.bass_utils as _bass_utils
from concourse.bass_utils import run_bass_kernel_spmd

# (walrus's --enable-ldw-opt pass was tried and rejects this kernel's
# DoubleRow self-loading matmuls; keep the default.)

F32 = mybir.dt.float32
BF16 = mybir.dt.bfloat16
F8 = mybir.dt.float8e4
AF = mybir.ActivationFunctionType
DR = mybir.MatmulPerfMode.DoubleRow
BF = ml_dtypes.bfloat16
E4 = ml_dtypes.float8_e4m3

B, N, BC = 16, 4096, 2          # batches, nodes, batches per core
NCORES = 8
NB = N // 512                   # 8 i-blocks of 512 output nodes
JT = N // 128                   # 32 contraction j-tiles
MAGIC = float(1.5 * 2 ** 23)    # fp32 round-to-nearest magic constant
TWO_PI = float(2.0 * np.pi)
ASCALE = 4096.0                 # adj premultiplier (host, before fp8 cast)
FSCALE = 16.0                   # f_cut premultiplier (device, before fp8)
OSCALE = float(1.0 / (ASCALE * FSCALE))

run_kwargs = {}                 # test.py may inject trace kwargs here


def split_excess_waits(nc, max_waits=1):
    """Walrus codegen on this image rejects >1 sem wait per instruction;
    move excess waits onto preceding same-engine no-ops."""
    n_split = 0
    for fn in nc.m.functions:
        for blk in fn.blocks:
            insts = list(blk.instructions)
            out = []
            changed = False
            for inst in insts:
                si = getattr(inst, "sync_info", None)
                if si is not None and len(si.on_wait) > max_waits:
                    waits = list(si.on_wait)
                    chunks = [waits[i:i + max_waits]
                              for i in range(0, len(waits), max_waits)]
                    for ci, ch in enumerate(chunks[:-1]):
                        nop = mybir.InstNoOp(
                            name=f"{inst.name}-wsplit-{ci}", ins=[], outs=[])
                        nop.engine = inst.engine
                        nop.sync_info = mybir.SyncInfo(on_wait=ch, on_update=[])
                        out.append(nop)
                        n_split += 1
                    inst.sync_info = mybir.SyncInfo(
                        on_wait=chunks[-1], on_update=list(si.on_update))
                    changed = True
                out.append(inst)
            if changed:
                blk.instructions = out
    return n_split


def _param(nc, name, shape, dt):
    return nc.declare_dram_parameter(name, list(shape), dt, isOutput=False)


def build_bass(split=True):
    nc = bass.Bass()

    adjT8 = _param(nc, "adjT8", [N, N], F8)
    meshHLd = _param(nc, "meshHLd", [14, N], BF16)
    onehotd = _param(nc, "onehotd", [BC, 64, N], BF16)
    maskTd = _param(nc, "maskTd", [50, BC], F32)

    pw1p = _param(nc, "pw1p", [67, 25], BF16)
    pw2d = _param(nc, "pw2d", [25, 50], BF16)
    w3t4d = _param(nc, "w3t4d", [114, 128], BF16)
    gw1d = _param(nc, "gw1d", [128, 128], BF16)
    gw2d = _param(nc, "gw2d", [128, 128], BF16)
    gw3d = _param(nc, "gw3d", [128, 50], BF16)
    aw1ad = _param(nc, "aw1ad", [50, 128], F32)
    aw1bd = _param(nc, "aw1bd", [50, 72], F32)
    aw2ad = _param(nc, "aw2ad", [128, 100], F32)
    aw2bd = _param(nc, "aw2bd", [72, 100], F32)
    aw3d = _param(nc, "aw3d", [100, 100], F32)
    gw0Ld = _param(nc, "gw0Ld", [100, 128], F32)
    pb3frd = _param(nc, "pb3frd", [1, 128], BF16)
    selfAd = _param(nc, "selfAd", [14, 128], BF16)
    selfBd = _param(nc, "selfBd", [14, 62], BF16)
    biasd = _param(nc, "biasd", [128, 12], F32)
    # bias columns (within biasd): 0 ab1a[128], 1 ab1b[72], 2 ab2[100],
    # 3 ab3[100], 4 pb3f[128], 5 pb1[25], 6 pb2[50], 7 gbl0[x2@0/64],
    # 8 gbl1, 9 gbl2, 10 gb3x2[114]
    outd = nc.declare_dram_parameter("outd", [114, 1], F32, isOutput=True)

    with tile.TileContext(nc) as tc:
        _emit(nc, tc, locals())
    if split:
        split_excess_waits(nc)
    return nc


def _emit(nc, tc, d):
    import contextlib
    ctx = contextlib.ExitStack()
    meshHLd, onehotd, maskTd = d["meshHLd"], d["onehotd"], d["maskTd"]
    biasd, outd = d["biasd"], d["outd"]

    cpool = ctx.enter_context(tc.tile_pool(name="consts", bufs=1))
    resp = ctx.enter_context(tc.tile_pool(name="resadj", bufs=1))
    actp = ctx.enter_context(tc.tile_pool(name="acts", bufs=1))
    smallp = ctx.enter_context(tc.tile_pool(name="small", bufs=2))
    dvep = ctx.enter_context(tc.tile_pool(name="dvework", bufs=3))
    h1p = ctx.enter_context(tc.tile_pool(name="h1p", bufs=2))

    # PSUM budget (8 banks): "feat"x2 + "bp"x2 + left0..left3 x1
    ps_a = ctx.enter_context(tc.tile_pool(name="psa", bufs=2, space="PSUM"))
    ps_b = ctx.enter_context(tc.tile_pool(name="psb", bufs=2, space="PSUM"))
    ps_c = ctx.enter_context(tc.tile_pool(name="psc", bufs=1, space="PSUM"))

    # ---------------- constants (small, issue before the big adj DMAs) ----
    def ctile(dram, shape, dt):
        nm = f"c_{dram.name}"
        t = cpool.tile(list(shape), dt, tag=nm, name=nm)
        nc.sync.dma_start(out=t[:], in_=dram[:])
        return t

    # issue order = arrival order: the action-MLP chain and the positional
    # front-end consume their params first, so their DMAs go first
    maskT = ctile(maskTd, [50, BC], F32)
    aw1a = ctile(d["aw1ad"], [50, 128], F32)
    aw1b = ctile(d["aw1bd"], [50, 72], F32)
    biases = ctile(biasd, [128, 12], F32)
    aw2a = ctile(d["aw2ad"], [128, 100], F32)
    aw2b = ctile(d["aw2bd"], [72, 100], F32)
    aw3 = ctile(d["aw3d"], [100, 100], F32)
    gw0L = ctile(d["gw0Ld"], [100, 128], F32)
    pb3fr = ctile(d["pb3frd"], [1, 128], BF16)
    selfA = ctile(d["selfAd"], [14, 128], BF16)
    selfB = ctile(d["selfBd"], [14, 62], BF16)
    pw1 = ctile(d["pw1p"], [67, 25], BF16)
    pw2 = ctile(d["pw2d"], [25, 50], BF16)
    w3t4 = ctile(d["w3t4d"], [114, 128], BF16)
    gws = [None, ctile(d["gw1d"], [128, 128], BF16),
           ctile(d["gw2d"], [128, 128], BF16),
           ctile(d["gw3d"], [128, 50], BF16)]
    ones1 = cpool.tile([1, 128], BF16, tag="ones1", name="ones1")
    nc.vector.memset(ones1[:], 1.0)

    def bcol(col, p0, p1):
        return biases[p0:p1, col:col + 1]

    # ---------------- resident adj^T (fp8, full) ----------------
    adjbig = resp.tile([128, JT * N], F8, tag="adj", name="adjbig")
    adj3 = adjbig[:].rearrange("p (q n) -> p q n", n=N)
    adjr = d["adjT8"][:].rearrange("(q p) c -> p q c", p=128)  # [128,32,4096]
    for q0 in range(0, JT, 4):
        nc.sync.dma_start(out=adj3[:, q0:q0 + 4, :], in_=adjr[:, q0:q0 + 4, :])

    # ---------------- activation tiles ----------------
    xt = actp.tile([128, BC * N], BF16, tag="x")          # [feat, b*N+n]
    # fcst padding columns 42:64 / 114:128 only feed PSUM partitions that
    # are never read back, but keep them zeroed: zero stationary columns
    # toggle less PE logic than garbage bits.
    fcst = actp.tile([128, JT * 128], F8, tag="fcst")     # node-major f_cut
    nc.scalar.memzero(fcst[:])
    fc3 = fcst[:].rearrange("p (q n) -> p q n", n=128)    # [128, 32, 128]
    # h2o: rows 0:4 one-hot, rows 4:64 zeros (alignment padding),
    # rows 64:114 h2 — one contraction for the layer-0 feature matmuls
    h2o = [actp.tile([114, N], BF16, tag=f"h2o{b}", name=f"h2o{b}")
           for b in range(BC)]
    nc.gpsimd.dma_start(out=h2o[0][0:64, :], in_=onehotd[0, :, :])
    nc.scalar.dma_start(out=h2o[1][0:64, :], in_=onehotd[1, :, :])
    cvec = actp.tile([128, BC], F32, tag="cvec")
    cvb = [actp.tile([1, 128], BF16, tag=f"cvb{b}", name=f"cvb{b}")
           for b in range(BC)]
    mx = actp.tile([114, NB], F32, tag="mx")
    outsb = actp.tile([114, 1], F32, tag="outsb")

    # ---------------- action MLP (tiny, fp32) ----------------
    pa = ps_a.tile([128, 2], F32, tag="feat")
    nc.tensor.matmul(pa[:], lhsT=aw1a[:], rhs=maskT[:], start=True, stop=True)
    a1a = smallp.tile([128, 2], F32, tag="a1a")
    nc.scalar.activation(a1a[:], pa[:], AF.Relu, bias=bcol(0, 0, 128))
    pb = ps_a.tile([72, 2], F32, tag="feat")
    nc.tensor.matmul(pb[:], lhsT=aw1b[:], rhs=maskT[:], start=True, stop=True)
    a1b = smallp.tile([72, 2], F32, tag="a1b")
    nc.scalar.activation(a1b[:], pb[:], AF.Relu, bias=bcol(1, 0, 72))
    pc = ps_a.tile([100, 2], F32, tag="feat")
    nc.tensor.matmul(pc[:], lhsT=aw2a[:], rhs=a1a[:], start=True, stop=False)
    nc.tensor.matmul(pc[:], lhsT=aw2b[:], rhs=a1b[:], start=False, stop=True)
    a2 = smallp.tile([100, 2], F32, tag="a2")
    nc.scalar.activation(a2[:], pc[:], AF.Relu, bias=bcol(2, 0, 100))
    pd = ps_a.tile([100, 2], F32, tag="feat")
    nc.tensor.matmul(pd[:], lhsT=aw3[:], rhs=a2[:], start=True, stop=True)
    a3 = smallp.tile([100, 2], F32, tag="a3")
    nc.scalar.activation(a3[:], pd[:], AF.Identity, bias=bcol(3, 0, 100))
    # cvec[f, b] = (a3 @ gw0[:100,:]) + pb3f  (bias for layer-0 features)
    pe_ = ps_a.tile([128, 2], F32, tag="feat")
    nc.tensor.matmul(pe_[:], lhsT=gw0L[:], rhs=a3[:], start=True, stop=True)
    nc.scalar.activation(cvec[:], pe_[:], AF.Identity, bias=bcol(4, 0, 128))
    # row versions cvb[b] = cvec[:, b].T for the node-major layer-0 matmul
    for b in range(BC):
        pr = ps_b.tile([1, 128], F32, tag="bp")
        nc.tensor.matmul(pr[:], lhsT=a3[:, b:b + 1], rhs=gw0L[:],
                         start=True, stop=False)
        nc.tensor.matmul(pr[:], lhsT=ones1[0:1, 0:1], rhs=pb3fr[:],
                         start=False, stop=True)
        nc.vector.tensor_copy(cvb[b][:], pr[:])
    # w3t4c[b] = w3t4 with the one-hot rows bumped by cvb[b]: folds the
    # action-embedding bias into the node-major layer-0 matmul
    w3t4c = [actp.tile([114, 128], BF16, tag=f"w3t4c{b}", name=f"w3t4c{b}")
             for b in range(BC)]
    for b in range(BC):
        pt4 = ps_b.tile([4, 128], F32, tag="bp")
        nc.tensor.matmul(pt4[:], lhsT=ones1[0:1, 0:4], rhs=cvb[b][0:1, :],
                         start=True, stop=True)
        nc.vector.tensor_copy(w3t4c[b][:], w3t4[:])
        nc.vector.tensor_add(w3t4c[b][0:4, :], w3t4[0:4, :], pt4[:])

    # ---------------- GCN layers ----------------
    def phase_bp(li, jt0, jt1):
        """Produce node-major fcst (fp8, x FSCALE): fc3[p, jt, 64b+c]."""
        cd = 42 if li < 3 else 50
        grp = 504 // cd  # jt groups per PSUM bank
        for b in range(BC):
            jt = jt0
            while jt < jt1:
                ng = min(grp, jt1 - jt)
                pg = ps_b.tile([128, grp * cd], F32, tag="bp")
                pg3 = pg[:].rearrange("p (g c) -> p g c", c=cd)
                for g in range(ng):
                    jc = slice((jt + g) * 128, (jt + g) * 128 + 128)
                    if li == 0:
                        nc.tensor.matmul(pg3[:, g, :], lhsT=h2o[b][:, jc],
                                         rhs=w3t4c[b][:, 0:cd],
                                         start=True, stop=True)
                    else:
                        nc.tensor.matmul(pg3[:, g, :],
                                         lhsT=xt[:, b * N + (jt + g) * 128:
                                                 b * N + (jt + g) * 128 + 128],
                                         rhs=gws[li][:, 0:cd],
                                         start=True, stop=True)
                dst = fc3[:, jt:jt + ng, 64 * b:64 * b + cd]
                src = pg3[:, 0:ng, :]
                if b == 0:
                    nc.vector.tensor_scalar_mul(dst, src, FSCALE)
                else:
                    nc.scalar.activation(dst, src, AF.Identity, scale=FSCALE)
                jt += ng

    def phase_a(li):
        """Feature-major right part: xt[32:128] = relu(f[32:128])."""
        for b in range(BC):
            for ch in range(NB):
                xs = slice(b * N + ch * 512, b * N + (ch + 1) * 512)
                pf = ps_a.tile([128, 512], F32, tag="feat")
                nc.tensor.matmul(pf[:], lhsT=gws[li][:],
                                 rhs=xt[:, xs], start=True, stop=True)
                nc.vector.tensor_scalar_max(xt[32:64, xs], pf[32:64, :],
                                            0.0)
                nc.scalar.activation(xt[64:128, xs], pf[64:128, :],
                                     AF.Relu)

    def c_alloc(half):
        return {ib: ps_c.tile([114, 512], F32, tag=f"left{ib % 4}",
                              name=f"left{ib % 4}") for ib in half}

    def c_half(li, pls, t0, t1):
        cd = 42 if li < 3 else 50
        W = 64 + cd
        for t in range(t0, t1):
            for ib in pls:
                nc.tensor.matmul(
                    pls[ib][0:W, :],
                    lhsT=fc3[:, 2 * t:2 * t + 2, 0:W],
                    rhs=adj3[:, 2 * t:2 * t + 2,
                             ib * 512:(ib + 1) * 512],
                    start=(t == 0), stop=(t == JT // 2 - 1),
                    perf_mode=DR)

    def c_acts(li, pls):
        if True:
            for ib in pls:
                if li < 3:
                    # xt[0:42] keeps the 2^16 (ASCALE*FSCALE) scale; the
                    # next layer's gw rows 0:42 are descaled on the host and
                    # the gb biases prescaled, so relu needs no scale here.
                    xs0 = slice(ib * 512, (ib + 1) * 512)
                    xs1 = slice(N + ib * 512, N + (ib + 1) * 512)
                    nc.vector.tensor_scalar(
                        xt[0:42, xs0], pls[ib][0:42, :],
                        bcol(7 + li, 0, 42), 0.0, ADD, MAX)
                    nc.scalar.activation(
                        xt[0:42, xs1], pls[ib][64:106, :], AF.Relu,
                        bias=bcol(7 + li, 64, 106))
                else:
                    # one 114-partition reduce; rows 50:64 are garbage but
                    # never read back on the host
                    nc.vector.tensor_reduce(
                        mx[:, ib:ib + 1], pls[ib][:, :],
                        mybir.AxisListType.X, mybir.AluOpType.max)


    # ---------------- positional front-end (+ layer-0 A/B\' interleave) ---
    # m6 rows: 0:6 mesh hi (b,c), 6:12 mesh lo, 12 = 0.25, 13 = 1.0 (all
    # host-built, one DMA).  psA rows 0:62 = t, rows 64:126 = t + MAGIC (the
    # PE accumulates the row-13 MAGIC term last, rounding t to the nearest
    # integer in fp32).  psB = (t + 0.25) + MAGIC.
    ADD, MAX = mybir.AluOpType.add, mybir.AluOpType.max
    peins = []
    for i in range(4):
        pt = cpool.tile([67, 512], BF16, tag=f"pein{i}", name=f"pein{i}")
        nc.vector.memset(pt[:], 0.0)
        peins.append(pt)
    m6s = [cpool.tile([14, 512], BF16, tag=f"m6_{i}", name=f"m6_{i}")
           for i in range(3)]
    # prefetch the first two mesh chunks before entering the loop
    for k in range(2):
        nc.gpsimd.dma_start(out=m6s[k][:],
                            in_=meshHLd[:, k * 512:(k + 1) * 512])

    def a0_chunk(b, ch):
        cs = slice(ch * 512, (ch + 1) * 512)
        xs = slice(b * N + ch * 512, b * N + (ch + 1) * 512)
        # borrow the (idle during phase 1) phase-C banks so the layer-0
        # feature matmuls don't serialize against the ph1/ph2 rotation
        pf = ps_c.tile([128, 512], F32, tag=f"left{b}", name=f"left{b}")
        nc.tensor.matmul(pf[:], lhsT=w3t4[:], rhs=h2o[b][:, cs],
                         start=True, stop=True)
        nc.vector.tensor_scalar(xt[32:64, xs], pf[32:64, :],
                                cvec[32:64, b:b + 1], 0.0, ADD, MAX)
        nc.scalar.activation(xt[64:128, xs], pf[64:128, :],
                             AF.Relu, bias=cvec[64:128, b:b + 1])

    for ch in range(NB):
        cs = slice(ch * 512, (ch + 1) * 512)
        m6 = m6s[ch % 3]
        if ch + 2 < NB:
            nc.gpsimd.dma_start(
                out=m6s[(ch + 2) % 3][:],
                in_=meshHLd[:, (ch + 2) * 512:(ch + 3) * 512])
        psA = ps_b.tile([128, 512], F32, tag="bp")
        nc.tensor.matmul(psA[:], lhsT=selfA[:], rhs=m6[:], start=True,
                         stop=True)
        psB = ps_b.tile([62, 512], F32, tag="bp")
        nc.tensor.matmul(psB[:], lhsT=selfB[:], rhs=m6[:], start=True,
                         stop=True)
        # rr rows 0:62 = round(t); rows 64:126 = round(t+0.25) (exact:
        # Sterbenz cancellation of MAGIC)
        rr = dvep.tile([128, 512], F32, tag="rr")
        nc.vector.tensor_scalar_add(rr[0:62, :], psA[64:126, :], -MAGIC)
        # rows 64:126 = round(t+0.25) - 0.25 (both subtractions exact)
        nc.vector.tensor_scalar(rr[64:126, :], psB[:], -MAGIC, -0.25,
                                ADD, ADD)
        # ddc rows 0:62 = t - round(t); rows 64:126 = (t+.25) - round(t+.25)
        ddc = dvep.tile([128, 512], F32, tag="ddc")
        nc.vector.tensor_sub(ddc[0:62, :], psA[0:62, :], rr[0:62, :])
        nc.vector.tensor_sub(ddc[64:126, :], psA[0:62, :], rr[64:126, :])
        for b in range(BC):
            pein = peins[2 * b + ch % 2]
            nc.scalar.activation(pein[0:30, :], ddc[32 * b:32 * b + 30, :],
                                 AF.Sin, scale=TWO_PI)
            nc.scalar.activation(pein[32:62, :],
                                 ddc[64 + 32 * b:64 + 32 * b + 30, :],
                                 AF.Sin, scale=TWO_PI)
            nc.gpsimd.dma_start(out=pein[64:67, :],
                                in_=meshHLd[3 * b:3 * b + 3, cs])
            # h1 = relu(pe_in @ pw1 + pb1)
            ph1 = ps_a.tile([25, 512], F32, tag="feat")
            nc.tensor.matmul(ph1[:], lhsT=pw1[:], rhs=pein[:],
                             start=True, stop=True)
            h1t = h1p.tile([25, 512], BF16, tag="h1")
            nc.scalar.activation(h1t[:], ph1[:], AF.Relu, bias=bcol(5, 0, 25))
            # h2 = relu(h1 @ pw2 + pb2)  (relu+bias on DVE to unload scalar)
            ph2 = ps_a.tile([50, 512], F32, tag="feat")
            nc.tensor.matmul(ph2[:], lhsT=pw2[:], rhs=h1t[:],
                             start=True, stop=True)
            nc.vector.tensor_scalar(h2o[b][64:114, cs], ph2[:],
                                    bcol(6, 0, 50), 0.0, ADD, MAX)
        for b in range(BC):
            a0_chunk(b, ch)
        if ch == 2:
            phase_bp(0, 0, 12)
        elif ch == 5:
            phase_bp(0, 12, 24)
        elif ch == 7:
            phase_bp(0, 24, JT)

    for li in (0, 1, 2, 3):
        if 0 < li < 3:
            phase_a(li)
        phase_bp(li, 0, JT)
        pls = c_alloc(range(0, 4))
        c_half(li, pls, 0, JT // 2)
        c_acts(li, pls)
        pls = c_alloc(range(4, NB))
        c_half(li, pls, 0, JT // 2)
        c_acts(li, pls)

    # ---------------- final max + bias + output ----------------
    mxr = smallp.tile([114, 1], F32, tag="mxr")
    nc.vector.tensor_reduce(mxr[:], mx[:], mybir.AxisListType.X,
                            mybir.AluOpType.max)
    nc.scalar.activation(outsb[:], mxr[:], AF.Identity, bias=bcol(10, 0, 114),
                         scale=OSCALE)
    nc.sync.dma_start(out=outd[:], in_=outsb[:])
    ctx.close()


# ---------------------------------------------------------------------------
# host side
# ---------------------------------------------------------------------------

def _descale_gw(gw):
    """Rows 0:42 consume the 2^16-scaled adjacency output; descale (exact)."""
    g = gw.astype(np.float32).copy()
    g[0:42] *= np.float32(1.0 / (ASCALE * FSCALE))
    return g.astype(BF)


def _prep_shared(inp):
    """Host preprocessing shared across cores (weights + adj)."""
    f32 = np.float32
    adjT8 = (np.ascontiguousarray(inp["adj"].astype(f32).T)
             * np.float32(ASCALE)).astype(E4)

    gw0 = inp["gw0"].astype(f32)
    w3fold = (inp["pw3"].astype(f32) @ gw0[100:200]).astype(BF)
    t4 = (inp["emb"].astype(f32) @ gw0[200:300]).astype(BF)
    pb3f = (inp["pb3"].astype(f32) @ gw0[100:200]).astype(f32)
    w3t4 = np.zeros((114, 128), BF)
    w3t4[0:4] = t4
    w3t4[64:114] = w3fold

    # pe_in row permutation: ours = [sin(f,c) x30 | cos(f,c) x30 | mesh x3]
    pw1f = inp["pw1"].astype(f32)
    pw1p_ = np.zeros((67, 25), f32)
    for k in range(30):
        f, c = divmod(k, 3)
        pw1p_[k] = pw1f[f * 6 + c]          # sin rows
        pw1p_[32 + k] = pw1f[f * 6 + 3 + c]  # cos rows
    pw1p_[64:67] = pw1f[60:63]
    pw1p = pw1p_.astype(BF)

    freqs = np.asarray([np.pi] + [2.0 * np.pi * i for i in range(1, 10)], f32)
    freq2 = np.repeat(freqs, 3) / (2.0 * np.pi)   # [30]
    self6 = np.zeros((6, 62), f32)
    for b in range(2):
        for k in range(30):
            self6[3 * b + k % 3, 32 * b + k] = freq2[k]
    # m6 rows: 0:6 mesh hi, 6:12 mesh lo, 12 = 0.25, 13 = 1.0.  selfA maps
    # them to t (cols 0:62) and t + MAGIC (cols 64:126); selfB to
    # (t + 0.25) + MAGIC.  freq2 entries and MAGIC are bf16-exact; the PE
    # accumulates rows in order, so the MAGIC term lands last and rounds
    # t (resp. t + 0.25) to the nearest integer in the fp32 accumulator.
    selfA = np.zeros((14, 128), f32)
    selfA[0:6, 0:62] = self6
    selfA[0:6, 64:126] = self6
    selfA[6:12, 0:62] = self6
    selfA[6:12, 64:126] = self6
    selfA[13, 64:126] = np.float32(MAGIC)
    selfB = np.zeros((14, 62), f32)
    selfB[0:6, :] = self6
    selfB[6:12, :] = self6
    selfB[12, :] = 1.0
    selfB[13, :] = np.float32(MAGIC)

    biasd = np.zeros((128, 12), f32)
    biasd[0:128, 0] = inp["ab1"][:128]
    biasd[0:72, 1] = inp["ab1"][128:200]
    biasd[0:100, 2] = inp["ab2"]
    biasd[0:100, 3] = inp["ab3"]
    biasd[0:128, 4] = pb3f
    biasd[0:25, 5] = inp["pb1"].astype(f32)
    biasd[0:50, 6] = inp["pb2"].astype(f32)
    # gb biases for layers 0-2 prescaled by 2^16 = ASCALE*FSCALE: the
    # post-adjacency relu output keeps that scale, and the next layer's gw
    # rows 0:42 are descaled to compensate.
    XS = np.float32(ASCALE * FSCALE)
    for li in range(3):
        biasd[0:42, 7 + li] = inp[f"gb{li}"].astype(f32)[:42] * XS
        biasd[64:106, 7 + li] = inp[f"gb{li}"].astype(f32)[:42] * XS
    biasd[0:50, 10] = inp["gb3"].astype(f32)
    biasd[64:114, 10] = inp["gb3"].astype(f32)

    return {
        "adjT8": adjT8,
        "pw1p": pw1p,
        "pw2d": inp["pw2"].astype(BF),
        "w3t4d": w3t4,
        "gw1d": _descale_gw(inp["gw1"]),
        "gw2d": _descale_gw(inp["gw2"]),
        "gw3d": _descale_gw(inp["gw3"]),
        "aw1ad": np.ascontiguousarray(inp["aw1"].astype(f32)[:, :128]),
        "aw1bd": np.ascontiguousarray(inp["aw1"].astype(f32)[:, 128:200]),
        "aw2ad": np.ascontiguousarray(inp["aw2"].astype(f32)[:128]),
        "aw2bd": np.ascontiguousarray(inp["aw2"].astype(f32)[128:200]),
        "aw3d": inp["aw3"].astype(f32),
        "gw0Ld": np.ascontiguousarray(gw0[:100]),
        "pb3frd": pb3f.reshape(1, 128).astype(BF),
        "selfAd": selfA.astype(BF),
        "selfBd": selfB.astype(BF),
        "biasd": biasd,
    }


def _prep_core(inp, shared, core):
    bs = slice(core * BC, (core + 1) * BC)
    f32 = np.float32
    mesh = inp["mesh"].astype(f32)[bs]                       # [2, N, 3]
    meshT = mesh.transpose(0, 2, 1).reshape(6, N)            # rows (b,c)
    hi = meshT.astype(BF)
    lo = (meshT - hi.astype(f32)).astype(BF)
    meshHL = np.zeros((14, N), BF)
    meshHL[0:6] = hi
    meshHL[6:12] = lo
    meshHL[12] = BF(0.25)
    meshHL[13] = BF(1.0)
    mi = inp["mask_idx"][bs]                                 # [2, N] int32
    oh = (mi[:, None, :] == np.arange(4, dtype=mi.dtype)[None, :, None])
    onehot = np.zeros((BC, 64, N), BF)
    onehot[:, 0:4, :] = oh.astype(BF)
    maskT = np.ascontiguousarray(inp["mask"].astype(f32)[bs].T)  # [50, 2]
    m = dict(shared)
    m["meshHLd"] = meshHL
    m["onehotd"] = onehot
    m["maskTd"] = maskT
    return m


_CACHED = {}


def kernel(**inputs) -> np.ndarray:
    if "nc" not in _CACHED:
        _CACHED["nc"] = build_bass()
    nc = _CACHED["nc"]
    shared = _prep_shared(inputs)
    in_maps = [_prep_core(inputs, shared, c) for c in range(NCORES)]
    res = run_bass_kernel_spmd(nc, in_maps, list(range(NCORES)), **run_kwargs)
    out = np.empty((B, 50), np.float32)
    for c in range(NCORES):
        o = res.results[c]["outd"][:, 0]
        out[2 * c] = o[0:50]
        out[2 * c + 1] = o[64:114]
    _CACHED["last_results"] = res
    return out


# revision 34
# speedup vs baseline: 1.0545x; 1.0019x over previous
"""Trainium2 Bass kernel for the GNN message-passing model.

Strategy: pure data-parallel over batch (B=16 -> 2 batches per core, 8 cores,
no cross-core communication). Activations are feature-major
([feat, batch*node]) for the per-node matmuls. The adjacency matmul keeps the
FULL adjacency SBUF-resident as fp8 e4m3 (adj^T * 4096, 16.8 MB) and runs in
DoubleRow fp8 perf mode (2 contraction rows/cycle): stationary operand is the
node-major cut-feature tile (fp8, scaled by 16), moving operand is a pair of
adj^T j-tiles. The node-major f_cut is produced directly by a second small
matmul per 128-node tile (stationary = x-slice, moving = cut columns of the
layer weight), avoiding PE transposes + PSUM casts. The 1/(4096*16) descale
is folded into the post-adjacency activation's scale.

Weight-only folds done on host (pure parameter preprocessing):
  W3fold = pw3 @ gw0[100:200]   (positional-MLP last layer folded into gw0)
  t4     = emb @ gw0[200:300]   (embedding table folded into gw0)
  pb3f   = pb3 @ gw0[100:200]   (bias fold)
mask_idx is re-encoded as a one-hot (4 classes, exact in fp8) so the
embedding lookup becomes a K=4 matmul accumulated into the same PSUM.
"""

import numpy as np
import ml_dtypes

import concourse.bass as bass
import concourse.mybir as mybir
import concourse.tile as tile
import concourse.bass_utils as _bass_utils
from concourse.bass_utils import run_bass_kernel_spmd

# (walrus's --enable-ldw-opt pass was tried and rejects this kernel's
# DoubleRow self-loading matmuls; keep the default.)

F32 = mybir.dt.float32
BF16 = mybir.dt.bfloat16
F8 = mybir.dt.float8e4
AF = mybir.ActivationFunctionType
DR = mybir.MatmulPerfMode.DoubleRow
BF = ml_dtypes.bfloat16
E4 = ml_dtypes.float8_e4m3

B, N, BC = 16, 4096, 2          # batches, nodes, batches per core
NCORES = 8
NB = N // 512                   # 8 i-blocks of 512 output nodes
JT = N // 128                   # 32 contraction j-tiles
MAGIC = float(1.5 * 2 ** 23)    # fp32 round-to-nearest magic constant
TWO_PI = float(2.0 * np.pi)
ASCALE = 4096.0                 # adj premultiplier (host, before fp8 cast)
FSCALE = 16.0                   # f_cut premultiplier (device, before fp8)
OSCALE = float(1.0 / (ASCALE * FSCALE))

run_kwargs = {}                 # test.py may inject trace kwargs here


def split_excess_waits(nc, max_waits=1):
    """Walrus codegen on this image rejects >1 sem wait per instruction;
    move excess waits onto preceding same-engine no-ops."""
    n_split = 0
    for fn in nc.m.functions:
        for blk in fn.blocks:
            insts = list(blk.instructions)
            out = []
            changed = False
            for inst in insts:
                si = getattr(inst, "sync_info", None)
                if si is not None and len(si.on_wait) > max_waits:
                    waits = list(si.on_wait)
                    chunks = [waits[i:i + max_waits]
                              for i in range(0, len(waits), max_waits)]
                    for ci, ch in enumerate(chunks[:-1]):
                        nop = mybir.InstNoOp(
                            name=f"{inst.name}-wsplit-{ci}", ins=[], outs=[])
                        nop.engine = inst.engine
                        nop.sync_info = mybir.SyncInfo(on_wait=ch, on_update=[])
                        out.append(nop)
                        n_split += 1
                    inst.sync_info = mybir.SyncInfo(
                        on_wait=chunks[-1], on_update=list(si.on_update))
                    changed = True
                out.append(inst)
            if changed:
                blk.instructions = out
    return n_split


def _param(nc, name, shape, dt):
    return nc.declare_dram_parameter(name, list(shape), dt, isOutput=False)


def build_bass(split=True):
    nc = bass.Bass()

    adjT8 = _param(nc, "adjT8", [N, N], F8)
    meshHLd = _param(nc, "meshHLd", [14, N], BF16)
    onehotd = _param(nc, "onehotd", [BC, 64, N], BF16)
    maskTd = _param(nc, "maskTd", [50, BC], F32)

    pw1p = _param(nc, "pw1p", [67, 25], BF16)
    pw2d = _param(nc, "pw2d", [25, 50], BF16)
    w3t4d = _param(nc, "w3t4d", [114, 128], BF16)
    gw1d = _param(nc, "gw1d", [128, 128], BF16)
    gw2d = _param(nc, "gw2d", [128, 128], BF16)
    gw3d = _param(nc, "gw3d", [128, 50], BF16)
    aw1ad = _param(nc, "aw1ad", [50, 128], F32)
    aw1bd = _param(nc, "aw1bd", [50, 72], F32)
    aw2ad = _param(nc, "aw2ad", [128, 100], F32)
    aw2bd = _param(nc, "aw2bd", [72, 100], F32)
    aw3d = _param(nc, "aw3d", [100, 100], F32)
    gw0Ld = _param(nc, "gw0Ld", [100, 128], F32)
    pb3frd = _param(nc, "pb3frd", [1, 128], BF16)
    selfAd = _param(nc, "selfAd", [14, 128], BF16)
    selfBd = _param(nc, "selfBd", [14, 62], BF16)
    biasd = _param(nc, "biasd", [128, 12], F32)
    # bias columns (within biasd): 0 ab1a[128], 1 ab1b[72], 2 ab2[100],
    # 3 ab3[100], 4 pb3f[128], 5 pb1[25], 6 pb2[50], 7 gbl0[x2@0/64],
    # 8 gbl1, 9 gbl2, 10 gb3x2[114]
    outd = nc.declare_dram_parameter("outd", [114, 1], F32, isOutput=True)

    with tile.TileContext(nc) as tc:
        _emit(nc, tc, locals())
    if split:
        split_excess_waits(nc)
    return nc


def _emit(nc, tc, d):
    import contextlib
    ctx = contextlib.ExitStack()
    meshHLd, onehotd, maskTd = d["meshHLd"], d["onehotd"], d["maskTd"]
    biasd, outd = d["biasd"], d["outd"]

    cpool = ctx.enter_context(tc.tile_pool(name="consts", bufs=1))
    resp = ctx.enter_context(tc.tile_pool(name="resadj", bufs=1))
    actp = ctx.enter_context(tc.tile_pool(name="acts", bufs=1))
    smallp = ctx.enter_context(tc.tile_pool(name="small", bufs=2))
    dvep = ctx.enter_context(tc.tile_pool(name="dvework", bufs=3))
    h1p = ctx.enter_context(tc.tile_pool(name="h1p", bufs=2))

    # PSUM budget (8 banks): "feat"x2 + "bp"x2 + left0..left3 x1
    ps_a = ctx.enter_context(tc.tile_pool(name="psa", bufs=2, space="PSUM"))
    ps_b = ctx.enter_context(tc.tile_pool(name="psb", bufs=2, space="PSUM"))
    ps_c = ctx.enter_context(tc.tile_pool(name="psc", bufs=1, space="PSUM"))

    # ---------------- constants (small, issue before the big adj DMAs) ----
    def ctile(dram, shape, dt):
        nm = f"c_{dram.name}"
        t = cpool.tile(list(shape), dt, tag=nm, name=nm)
        nc.sync.dma_start(out=t[:], in_=dram[:])
        return t

    # issue order = arrival order: the action-MLP chain and the positional
    # front-end consume their params first, so their DMAs go first
    maskT = ctile(maskTd, [50, BC], F32)
    aw1a = ctile(d["aw1ad"], [50, 128], F32)
    aw1b = ctile(d["aw1bd"], [50, 72], F32)
    biases = ctile(biasd, [128, 12], F32)
    aw2a = ctile(d["aw2ad"], [128, 100], F32)
    aw2b = ctile(d["aw2bd"], [72, 100], F32)
    aw3 = ctile(d["aw3d"], [100, 100], F32)
    gw0L = ctile(d["gw0Ld"], [100, 128], F32)
    pb3fr = ctile(d["pb3frd"], [1, 128], BF16)
    selfA = ctile(d["selfAd"], [14, 128], BF16)
    selfB = ctile(d["selfBd"], [14, 62], BF16)
    pw1 = ctile(d["pw1p"], [67, 25], BF16)
    pw2 = ctile(d["pw2d"], [25, 50], BF16)
    w3t4 = ctile(d["w3t4d"], [114, 128], BF16)
    gws = [None, ctile(d["gw1d"], [128, 128], BF16),
           ctile(d["gw2d"], [128, 128], BF16),
           ctile(d["gw3d"], [128, 50], BF16)]
    ones1 = cpool.tile([1, 128], BF16, tag="ones1", name="ones1")
    nc.vector.memset(ones1[:], 1.0)

    def bcol(col, p0, p1):
        return biases[p0:p1, col:col + 1]

    # ---------------- resident adj^T (fp8, full) ----------------
    adjbig = resp.tile([128, JT * N], F8, tag="adj", name="adjbig")
    adj3 = adjbig[:].rearrange("p (q n) -> p q n", n=N)
    adjr = d["adjT8"][:].rearrange("(q p) c -> p q c", p=128)  # [128,32,4096]
    for q0 in range(0, JT, 4):
        nc.sync.dma_start(out=adj3[:, q0:q0 + 4, :], in_=adjr[:, q0:q0 + 4, :])

    # ---------------- activation tiles ----------------
    xt = actp.tile([128, BC * N], BF16, tag="x")          # [feat, b*N+n]
    # fcst padding columns 42:64 / 114:128 only feed PSUM partitions that
    # are never read back, but keep them zeroed: zero stationary columns
    # toggle less PE logic than garbage bits.
    fcst = actp.tile([128, JT * 128], F8, tag="fcst")     # node-major f_cut
    nc.scalar.memzero(fcst[:])
    fc3 = fcst[:].rearrange("p (q n) -> p q n", n=128)    # [128, 32, 128]
    # h2o: rows 0:4 one-hot, rows 4:64 zeros (alignment padding),
    # rows 64:114 h2 — one contraction for the layer-0 feature matmuls
    h2o = [actp.tile([114, N], BF16, tag=f"h2o{b}", name=f"h2o{b}")
           for b in range(BC)]
    nc.gpsimd.dma_start(out=h2o[0][0:64, :], in_=onehotd[0, :, :])
    nc.scalar.dma_start(out=h2o[1][0:64, :], in_=onehotd[1, :, :])
    cvec = actp.tile([128, BC], F32, tag="cvec")
    cvb = [actp.tile([1, 128], BF16, tag=f"cvb{b}", name=f"cvb{b}")
           for b in range(BC)]
    mx = actp.tile([114, NB], F32, tag="mx")
    outsb = actp.tile([114, 1], F32, tag="outsb")

    # ---------------- action MLP (tiny, fp32) ----------------
    pa = ps_a.tile([128, 2], F32, tag="feat")
    nc.tensor.matmul(pa[:], lhsT=aw1a[:], rhs=maskT[:], start=True, stop=True)
    a1a = smallp.tile([128, 2], F32, tag="a1a")
    nc.scalar.activation(a1a[:], pa[:], AF.Relu, bias=bcol(0, 0, 128))
    pb = ps_a.tile([72, 2], F32, tag="feat")
    nc.tensor.matmul(pb[:], lhsT=aw1b[:], rhs=maskT[:], start=True, stop=True)
    a1b = smallp.tile([72, 2], F32, tag="a1b")
    nc.scalar.activation(a1b[:], pb[:], AF.Relu, bias=bcol(1, 0, 72))
    pc = ps_a.tile([100, 2], F32, tag="feat")
    nc.tensor.matmul(pc[:], lhsT=aw2a[:], rhs=a1a[:], start=True, stop=False)
    nc.tensor.matmul(pc[:], lhsT=aw2b[:], rhs=a1b[:], start=False, stop=True)
    a2 = smallp.tile([100, 2], F32, tag="a2")
    nc.scalar.activation(a2[:], pc[:], AF.Relu, bias=bcol(2, 0, 100))
    pd = ps_a.tile([100, 2], F32, tag="feat")
    nc.tensor.matmul(pd[:], lhsT=aw3[:], rhs=a2[:], start=True, stop=True)
    a3 = smallp.tile([100, 2], F32, tag="a3")
    nc.scalar.activation(a3[:], pd[:], AF.Identity, bias=bcol(3, 0, 100))
    # cvec[f, b] = (a3 @ gw0[:100,:]) + pb3f  (bias for layer-0 features)
    pe_ = ps_a.tile([128, 2], F32, tag="feat")
    nc.tensor.matmul(pe_[:], lhsT=gw0L[:], rhs=a3[:], start=True, stop=True)
    nc.scalar.activation(cvec[:], pe_[:], AF.Identity, bias=bcol(4, 0, 128))
    # row versions cvb[b] = cvec[:, b].T for the node-major layer-0 matmul
    for b in range(BC):
        pr = ps_b.tile([1, 128], F32, tag="bp")
        nc.tensor.matmul(pr[:], lhsT=a3[:, b:b + 1], rhs=gw0L[:],
                         start=True, stop=False)
        nc.tensor.matmul(pr[:], lhsT=ones1[0:1, 0:1], rhs=pb3fr[:],
                         start=False, stop=True)
        nc.vector.tensor_copy(cvb[b][:], pr[:])
    # w3t4c[b] = w3t4 with the one-hot rows bumped by cvb[b]: folds the
    # action-embedding bias into the node-major layer-0 matmul
    w3t4c = [actp.tile([114, 128], BF16, tag=f"w3t4c{b}", name=f"w3t4c{b}")
             for b in range(BC)]
    for b in range(BC):
        pt4 = ps_b.tile([4, 128], F32, tag="bp")
        nc.tensor.matmul(pt4[:], lhsT=ones1[0:1, 0:4], rhs=cvb[b][0:1, :],
                         start=True, stop=True)
        nc.vector.tensor_copy(w3t4c[b][:], w3t4[:])
        nc.vector.tensor_add(w3t4c[b][0:4, :], w3t4[0:4, :], pt4[:])

    # ---------------- GCN layers ----------------
    def phase_bp(li, jt0, jt1):
        """Produce node-major fcst (fp8, x FSCALE): fc3[p, jt, 64b+c]."""
        cd = 42 if li < 3 else 50
        grp = 504 // cd  # jt groups per PSUM bank
        for b in range(BC):
            jt = jt0
            while jt < jt1:
                ng = min(grp, jt1 - jt)
                pg = ps_b.tile([128, grp * cd], F32, tag="bp")
                pg3 = pg[:].rearrange("p (g c) -> p g c", c=cd)
                for g in range(ng):
                    jc = slice((jt + g) * 128, (jt + g) * 128 + 128)
                    if li == 0:
                        nc.tensor.matmul(pg3[:, g, :], lhsT=h2o[b][:, jc],
                                         rhs=w3t4c[b][:, 0:cd],
                                         start=True, stop=True)
                    else:
                        nc.tensor.matmul(pg3[:, g, :],
                                         lhsT=xt[:, b * N + (jt + g) * 128:
                                                 b * N + (jt + g) * 128 + 128],
                                         rhs=gws[li][:, 0:cd],
                                         start=True, stop=True)
                dst = fc3[:, jt:jt + ng, 64 * b:64 * b + cd]
                src = pg3[:, 0:ng, :]
                if b == 0:
                    nc.vector.tensor_scalar_mul(dst, src, FSCALE)
                else:
                    nc.scalar.activation(dst, src, AF.Identity, scale=FSCALE)
                jt += ng

    def phase_a(li):
        """Feature-major right part: xt[32:128] = relu(f[32:128])."""
        for b in range(BC):
            for ch in range(NB):
                xs = slice(b * N + ch * 512, b * N + (ch + 1) * 512)
                pf = ps_a.tile([128, 512], F32, tag="feat")
                nc.tensor.matmul(pf[:], lhsT=gws[li][:],
                                 rhs=xt[:, xs], start=True, stop=True)
                nc.vector.tensor_scalar_max(xt[32:64, xs], pf[32:64, :],
                                            0.0)
                nc.scalar.activation(xt[64:128, xs], pf[64:128, :],
                                     AF.Relu)

    def c_alloc(half):
        return {ib: ps_c.tile([114, 512], F32, tag=f"left{ib % 4}",
                              name=f"left{ib % 4}") for ib in half}

    def c_half(li, pls, t0, t1):
        cd = 42 if li < 3 else 50
        W = 64 + cd
        for t in range(t0, t1):
            for ib in pls:
                nc.tensor.matmul(
                    pls[ib][0:W, :],
                    lhsT=fc3[:, 2 * t:2 * t + 2, 0:W],
                    rhs=adj3[:, 2 * t:2 * t + 2,
                             ib * 512:(ib + 1) * 512],
                    start=(t == 0), stop=(t == JT // 2 - 1),
                    perf_mode=DR)

    def c_acts(li, pls):
        if True:
            for ib in pls:
                if li < 3:
                    # xt[0:42] keeps the 2^16 (ASCALE*FSCALE) scale; the
                    # next layer's gw rows 0:42 are descaled on the host and
                    # the gb biases prescaled, so relu needs no scale here.
                    xs0 = slice(ib * 512, (ib + 1) * 512)
                    xs1 = slice(N + ib * 512, N + (ib + 1) * 512)
                    nc.vector.tensor_scalar(
                        xt[0:42, xs0], pls[ib][0:42, :],
                        bcol(7 + li, 0, 42), 0.0, ADD, MAX)
                    nc.scalar.activation(
                        xt[0:42, xs1], pls[ib][64:106, :], AF.Relu,
                        bias=bcol(7 + li, 64, 106))
                else:
                    # one 114-partition reduce; rows 50:64 are garbage but
                    # never read back on the host
                    nc.vector.tensor_reduce(
                        mx[:, ib:ib + 1], pls[ib][:, :],
                        mybir.AxisListType.X, mybir.AluOpType.max)


    # ---------------- positional front-end (+ layer-0 A/B\' interleave) ---
    # m6 rows: 0:6 mesh hi (b,c), 6:12 mesh lo, 12 = 0.25, 13 = 1.0 (all
    # host-built, one DMA).  psA rows 0:62 = t, rows 64:126 = t + MAGIC (the
    # PE accumulates the row-13 MAGIC term last, rounding t to the nearest
    # integer in fp32).  psB = (t + 0.25) + MAGIC.
    ADD, MAX = mybir.AluOpType.add, mybir.AluOpType.max
    peins = []
    for i in range(4):
        pt = cpool.tile([67, 512], BF16, tag=f"pein{i}", name=f"pein{i}")
        nc.vector.memset(pt[:], 0.0)
        peins.append(pt)
    m6s = [cpool.tile([14, 512], BF16, tag=f"m6_{i}", name=f"m6_{i}")
           for i in range(3)]
    # prefetch the first two mesh chunks before entering the loop
    for k in range(2):
        nc.gpsimd.dma_start(out=m6s[k][:],
                            in_=meshHLd[:, k * 512:(k + 1) * 512])

    def a0_chunk(b, ch):
        cs = slice(ch * 512, (ch + 1) * 512)
        xs = slice(b * N + ch * 512, b * N + (ch + 1) * 512)
        # borrow the (idle during phase 1) phase-C banks so the layer-0
        # feature matmuls don't serialize against the ph1/ph2 rotation
        pf = ps_c.tile([128, 512], F32, tag=f"left{b}", name=f"left{b}")
        nc.tensor.matmul(pf[:], lhsT=w3t4[:], rhs=h2o[b][:, cs],
                         start=True, stop=True)
        nc.vector.tensor_scalar(xt[32:64, xs], pf[32:64, :],
                                cvec[32:64, b:b + 1], 0.0, ADD, MAX)
        nc.scalar.activation(xt[64:128, xs], pf[64:128, :],
                             AF.Relu, bias=cvec[64:128, b:b + 1])

    for ch in range(NB):
        cs = slice(ch * 512, (ch + 1) * 512)
        m6 = m6s[ch % 3]
        if ch + 2 < NB:
            nc.gpsimd.dma_start(
                out=m6s[(ch + 2) % 3][:],
                in_=meshHLd[:, (ch + 2) * 512:(ch + 3) * 512])
        psA = ps_b.tile([128, 512], F32, tag="bp")
        nc.tensor.matmul(psA[:], lhsT=selfA[:], rhs=m6[:], start=True,
                         stop=True)
        psB = ps_b.tile([62, 512], F32, tag="bp")
        nc.tensor.matmul(psB[:], lhsT=selfB[:], rhs=m6[:], start=True,
                         stop=True)
        # rr rows 0:62 = round(t); rows 64:126 = round(t+0.25) (exact:
        # Sterbenz cancellation of MAGIC)
        rr = dvep.tile([128, 512], F32, tag="rr")
        nc.vector.tensor_scalar_add(rr[0:62, :], psA[64:126, :], -MAGIC)
        # rows 64:126 = round(t+0.25) - 0.25 (both subtractions exact)
        nc.vector.tensor_scalar(rr[64:126, :], psB[:], -MAGIC, -0.25,
                                ADD, ADD)
        # ddc rows 0:62 = t - round(t); rows 64:126 = (t+.25) - round(t+.25)
        ddc = dvep.tile([128, 512], F32, tag="ddc")
        nc.vector.tensor_sub(ddc[0:62, :], psA[0:62, :], rr[0:62, :])
        nc.vector.tensor_sub(ddc[64:126, :], psA[0:62, :], rr[64:126, :])
        for b in range(BC):
            pein = peins[2 * b + ch % 2]
            nc.scalar.activation(pein[0:30, :], ddc[32 * b:32 * b + 30, :],
                                 AF.Sin, scale=TWO_PI)
            nc.scalar.activation(pein[32:62, :],
                                 ddc[64 + 32 * b:64 + 32 * b + 30, :],
                                 AF.Sin, scale=TWO_PI)
            nc.gpsimd.dma_start(out=pein[64:67, :],
                                in_=meshHLd[3 * b:3 * b + 3, cs])
            # h1 = relu(pe_in @ pw1 + pb1).  ph1/ph2 borrow the phase-C
            # banks that are idle during phase 1 (left2/left3; a0 uses
            # left0/left1) so they rotate against their own immediate
            # consumers instead of weaving through the shared "feat" pair.
            ph1 = ps_c.tile([25, 512], F32, tag="left2", name="left2")
            nc.tensor.matmul(ph1[:], lhsT=pw1[:], rhs=pein[:],
                             start=True, stop=True)
            h1t = h1p.tile([25, 512], BF16, tag="h1")
            nc.scalar.activation(h1t[:], ph1[:], AF.Relu, bias=bcol(5, 0, 25))
            # h2 = relu(h1 @ pw2 + pb2)  (relu+bias on DVE to unload scalar)
            ph2 = ps_c.tile([50, 512], F32, tag="left3", name="left3")
            nc.tensor.matmul(ph2[:], lhsT=pw2[:], rhs=h1t[:],
                             start=True, stop=True)
            nc.vector.tensor_scalar(h2o[b][64:114, cs], ph2[:],
                                    bcol(6, 0, 50), 0.0, ADD, MAX)
        for b in range(BC):
            a0_chunk(b, ch)
        if ch == 2:
            phase_bp(0, 0, 12)
        elif ch == 5:
            phase_bp(0, 12, 24)
        elif ch == 7:
            phase_bp(0, 24, JT)

    for li in (0, 1, 2, 3):
        if 0 < li < 3:
            phase_a(li)
        phase_bp(li, 0, JT)
        pls = c_alloc(range(0, 4))
        c_half(li, pls, 0, JT // 2)
        c_acts(li, pls)
        pls = c_alloc(range(4, NB))
        c_half(li, pls, 0, JT // 2)
        c_acts(li, pls)

    # ---------------- final max + bias + output ----------------
    mxr = smallp.tile([114, 1], F32, tag="mxr")
    nc.vector.tensor_reduce(mxr[:], mx[:], mybir.AxisListType.X,
                            mybir.AluOpType.max)
    nc.scalar.activation(outsb[:], mxr[:], AF.Identity, bias=bcol(10, 0, 114),
                         scale=OSCALE)
    nc.sync.dma_start(out=outd[:], in_=outsb[:])
    ctx.close()


# ---------------------------------------------------------------------------
# host side
# ---------------------------------------------------------------------------

def _descale_gw(gw):
    """Rows 0:42 consume the 2^16-scaled adjacency output; descale (exact)."""
    g = gw.astype(np.float32).copy()
    g[0:42] *= np.float32(1.0 / (ASCALE * FSCALE))
    return g.astype(BF)


def _prep_shared(inp):
    """Host preprocessing shared across cores (weights + adj)."""
    f32 = np.float32
    adjT8 = (np.ascontiguousarray(inp["adj"].astype(f32).T)
             * np.float32(ASCALE)).astype(E4)

    gw0 = inp["gw0"].astype(f32)
    w3fold = (inp["pw3"].astype(f32) @ gw0[100:200]).astype(BF)
    t4 = (inp["emb"].astype(f32) @ gw0[200:300]).astype(BF)
    pb3f = (inp["pb3"].astype(f32) @ gw0[100:200]).astype(f32)
    w3t4 = np.zeros((114, 128), BF)
    w3t4[0:4] = t4
    w3t4[64:114] = w3fold

    # pe_in row permutation: ours = [sin(f,c) x30 | cos(f,c) x30 | mesh x3]
    pw1f = inp["pw1"].astype(f32)
    pw1p_ = np.zeros((67, 25), f32)
    for k in range(30):
        f, c = divmod(k, 3)
        pw1p_[k] = pw1f[f * 6 + c]          # sin rows
        pw1p_[32 + k] = pw1f[f * 6 + 3 + c]  # cos rows
    pw1p_[64:67] = pw1f[60:63]
    pw1p = pw1p_.astype(BF)

    freqs = np.asarray([np.pi] + [2.0 * np.pi * i for i in range(1, 10)], f32)
    freq2 = np.repeat(freqs, 3) / (2.0 * np.pi)   # [30]
    self6 = np.zeros((6, 62), f32)
    for b in range(2):
        for k in range(30):
            self6[3 * b + k % 3, 32 * b + k] = freq2[k]
    # m6 rows: 0:6 mesh hi, 6:12 mesh lo, 12 = 0.25, 13 = 1.0.  selfA maps
    # them to t (cols 0:62) and t + MAGIC (cols 64:126); selfB to
    # (t + 0.25) + MAGIC.  freq2 entries and MAGIC are bf16-exact; the PE
    # accumulates rows in order, so the MAGIC term lands last and rounds
    # t (resp. t + 0.25) to the nearest integer in the fp32 accumulator.
    selfA = np.zeros((14, 128), f32)
    selfA[0:6, 0:62] = self6
    selfA[0:6, 64:126] = self6
    selfA[6:12, 0:62] = self6
    selfA[6:12, 64:126] = self6
    selfA[13, 64:126] = np.float32(MAGIC)
    selfB = np.zeros((14, 62), f32)
    selfB[0:6, :] = self6
    selfB[6:12, :] = self6
    selfB[12, :] = 1.0
    selfB[13, :] = np.float32(MAGIC)

    biasd = np.zeros((128, 12), f32)
    biasd[0:128, 0] = inp["ab1"][:128]
    biasd[0:72, 1] = inp["ab1"][128:200]
    biasd[0:100, 2] = inp["ab2"]
    biasd[0:100, 3] = inp["ab3"]
    biasd[0:128, 4] = pb3f
    biasd[0:25, 5] = inp["pb1"].astype(f32)
    biasd[0:50, 6] = inp["pb2"].astype(f32)
    # gb biases for layers 0-2 prescaled by 2^16 = ASCALE*FSCALE: the
    # post-adjacency relu output keeps that scale, and the next layer's gw
    # rows 0:42 are descaled to compensate.
    XS = np.float32(ASCALE * FSCALE)
    for li in range(3):
        biasd[0:42, 7 + li] = inp[f"gb{li}"].astype(f32)[:42] * XS
        biasd[64:106, 7 + li] = inp[f"gb{li}"].astype(f32)[:42] * XS
    biasd[0:50, 10] = inp["gb3"].astype(f32)
    biasd[64:114, 10] = inp["gb3"].astype(f32)

    return {
        "adjT8": adjT8,
        "pw1p": pw1p,
        "pw2d": inp["pw2"].astype(BF),
        "w3t4d": w3t4,
        "gw1d": _descale_gw(inp["gw1"]),
        "gw2d": _descale_gw(inp["gw2"]),
        "gw3d": _descale_gw(inp["gw3"]),
        "aw1ad": np.ascontiguousarray(inp["aw1"].astype(f32)[:, :128]),
        "aw1bd": np.ascontiguousarray(inp["aw1"].astype(f32)[:, 128:200]),
        "aw2ad": np.ascontiguousarray(inp["aw2"].astype(f32)[:128]),
        "aw2bd": np.ascontiguousarray(inp["aw2"].astype(f32)[128:200]),
        "aw3d": inp["aw3"].astype(f32),
        "gw0Ld": np.ascontiguousarray(gw0[:100]),
        "pb3frd": pb3f.reshape(1, 128).astype(BF),
        "selfAd": selfA.astype(BF),
        "selfBd": selfB.astype(BF),
        "biasd": biasd,
    }


def _prep_core(inp, shared, core):
    bs = slice(core * BC, (core + 1) * BC)
    f32 = np.float32
    mesh = inp["mesh"].astype(f32)[bs]                       # [2, N, 3]
    meshT = mesh.transpose(0, 2, 1).reshape(6, N)            # rows (b,c)
    hi = meshT.astype(BF)
    lo = (meshT - hi.astype(f32)).astype(BF)
    meshHL = np.zeros((14, N), BF)
    meshHL[0:6] = hi
    meshHL[6:12] = lo
    meshHL[12] = BF(0.25)
    meshHL[13] = BF(1.0)
    mi = inp["mask_idx"][bs]                                 # [2, N] int32
    oh = (mi[:, None, :] == np.arange(4, dtype=mi.dtype)[None, :, None])
    onehot = np.zeros((BC, 64, N), BF)
    onehot[:, 0:4, :] = oh.astype(BF)
    maskT = np.ascontiguousarray(inp["mask"].astype(f32)[bs].T)  # [50, 2]
    m = dict(shared)
    m["meshHLd"] = meshHL
    m["onehotd"] = onehot
    m["maskTd"] = maskT
    return m


_CACHED = {}


def kernel(**inputs) -> np.ndarray:
    if "nc" not in _CACHED:
        _CACHED["nc"] = build_bass()
    nc = _CACHED["nc"]
    shared = _prep_shared(inputs)
    in_maps = [_prep_core(inputs, shared, c) for c in range(NCORES)]
    res = run_bass_kernel_spmd(nc, in_maps, list(range(NCORES)), **run_kwargs)
    out = np.empty((B, 50), np.float32)
    for c in range(NCORES):
        o = res.results[c]["outd"][:, 0]
        out[2 * c] = o[0:50]
        out[2 * c + 1] = o[64:114]
    _CACHED["last_results"] = res
    return out
